# revision 8
# baseline (speedup 1.0000x reference)
"""GCN (2-layer) Trainium2 kernel over 8 NeuronCores.

Strategy:
- GCN is node-permutation-equivariant: the host renumbers nodes so that
  (a) each core owns 6250 nodes (padded shard 6272 = 49 tiles x 128),
  (b) in-edge counts are balanced so that every (dst-tile, src-half) bucket
      holds <= 1024 edges -> exactly 8 gather-blocks of 128, no max-over-core
      padding.  This minimizes the GPSIMD SWDGE descriptor-generation time,
      which is the hard serial bottleneck (~8ns/edge on the Pool engine).
- h1 = (x @ W1) scaled by dinv (deg^-1/2) computed shard-local -> AllGather
  to a full 50176-row table in each core's DRAM.
- Scatter-add aggregation out[d] += table[src] over REAL edges only is done
  per dst-core: rows fetched with gpsimd.dma_gather (int16 idx over two
  25088-row halves); segmented sum per 128-dst tile via TensorE matmul with
  a DVE-built one-hot selector.  The self-loop term is added locally from
  the resident shard (no gather traffic).
- Layer 2 aggregates the dinv-scaled relu table (64 feats), then W2 + bias +
  log_softmax on-chip.
The edge structure is baked into the program at build time (SPMD; identical
program on all 8 cores, per-core data differs, padded to a common shape).
"""

import numpy as np

N_NODES = 50000
CORES = 8
SH = 6250          # owned nodes per core
SHP = 6272         # padded shard rows (49*128)
NT = 49            # dst tiles per core
HALF = SHP * 4     # 25088 table rows per half (cores 0-3 | 4-7)
F0, F1, F2 = 96, 64, 16
BLK = 128
CHUNK_BLOCKS = 8   # 1024 idx per dma_gather (single_packet limit)
CHUNK = BLK * CHUNK_BLOCKS
ZROW = 25087       # half-local row of a guaranteed-zero table row (both halves)


# --------------------------------------------------------------------------
# host-side balancing: nodes -> (core, tile, slot)
# --------------------------------------------------------------------------

def _balance(src, dst):
    """Assign nodes to cores/tiles/slots, balancing real-edge in-degrees so
    that each (core, tile, src-half) bucket holds close to 1024 edges."""
    N = N_NODES
    deg = np.bincount(dst, minlength=N).astype(np.int64)

    # ---- phase 1: core assignment, balance total in-degree, cap 6250 ----
    order = np.argsort(-deg, kind="stable")
    core_of = np.empty(N, np.int64)
    load = np.zeros(CORES, np.int64)
    cnt = np.zeros(CORES, np.int64)
    for v in order:
        c = -1
        best = None
        for k in range(CORES):
            if cnt[k] < SH and (best is None or load[k] < best):
                best = load[k]
                c = k
        core_of[v] = c
        load[c] += deg[v]
        cnt[c] += 1

    # ---- phase 2: balance A/B half split of each core's in-edges --------
    # O[u, c] = # out-edges of u landing in core c
    O = np.zeros((N, CORES), np.int32)
    np.add.at(O, (src, core_of[dst]), 1)
    in_tot = load.copy()  # in-edges per core

    def half_excess():
        # E(c, A) - in(c)/2 for each core c (A = src core < 4)
        srcA = core_of[src] < 4
        EA = np.bincount(core_of[dst][srcA], minlength=CORES)
        return EA - in_tot / 2.0

    delta = half_excess()
    isA = core_of < 4
    for _ in range(400):
        w = int(np.argmax(np.abs(delta)))
        if abs(delta[w]) <= 16:
            break
        # delta[w] > 0: too many in-edges of w from half A -> move a node
        # with many out-edges-into-w from A to B (swap with equal-degree).
        if delta[w] > 0:
            cand1 = np.where(isA)[0]
            cand2mask = ~isA
        else:
            cand1 = np.where(~isA)[0]
            cand2mask = isA
        u1 = cand1[np.argmax(O[cand1, w])]
        d1 = deg[u1]
        cand2 = np.where(cand2mask & (deg == d1))[0]
        if len(cand2) == 0:
            break
        u2 = cand2[np.argmin(O[cand2, w])]
        # swap cores of u1, u2 (equal in-degree keeps in_tot intact)
        c1, c2 = core_of[u1], core_of[u2]
        core_of[u1], core_of[u2] = c2, c1
        isA[u1], isA[u2] = c2 < 4, c1 < 4
        # update delta incrementally: u1 moved A->B (or B->A), u2 opposite
        s1 = 1.0 if c1 < 4 else -1.0  # u1 leaves half(c1)
        delta = delta - s1 * O[u1] + s1 * O[u2]

    # ---- phase 3: per-core tile packing, 2D (dA, dB) <= (1024, 1024) ----
    srcA = core_of[src] < 4
    dA = np.bincount(dst[srcA], minlength=N).astype(np.int64)
    dB = deg - dA

    tile_of = np.empty(N, np.int64)
    slot_of = np.empty(N, np.int64)
    for c in range(CORES):
        nodes = np.where(core_of == c)[0]
        nodes = nodes[np.argsort(-(dA[nodes] + dB[nodes]), kind="stable")]
        cap = np.full(NT, BLK, np.int64)
        if c in (3, 7):
            cap[NT - 1] = BLK - 1  # reserve the ZROW slot
        la = np.zeros(NT, np.int64)
        lb = np.zeros(NT, np.int64)
        nc_ = np.zeros(NT, np.int64)
        tl = np.empty(len(nodes), np.int64)
        for i, v in enumerate(nodes):
            score = np.maximum(la + dA[v], lb + dB[v]).astype(np.float64)
            score[nc_ >= cap] = np.inf
            t = int(np.argmin(score))
            tl[i] = t
            la[t] += dA[v]
            lb[t] += dB[v]
            nc_[t] += 1
        # repair: move nodes out of overfull bins (either half > 1024)
        for _ in range(600):
            over = np.where((la > CHUNK) | (lb > CHUNK))[0]
            if len(over) == 0:
                break
            t = int(over[0])
            halfsel = la if la[t] > CHUNK else lb
            dsel = dA if la[t] > CHUNK else dB
            members = np.where(tl == t)[0]
            excess = halfsel[t] - CHUNK
            diffs = dsel[nodes[members]]
            k = members[np.argmin(np.abs(diffs - excess))]
            v = nodes[k]
            score = np.maximum(la + dA[v], lb + dB[v]).astype(np.float64)
            score[nc_ >= cap] = np.inf
            score[t] = np.inf
            t2 = int(np.argmin(score))
            tl[k] = t2
            la[t] -= dA[v]; lb[t] -= dB[v]; nc_[t] -= 1
            la[t2] += dA[v]; lb[t2] += dB[v]; nc_[t2] += 1
        # swap repair: exchange nodes between overfull/underfull tiles
        for _ in range(400):
            over = np.where((la > CHUNK) | (lb > CHUNK))[0]
            if len(over) == 0:
                break
            t = int(over[0])
            useA = la[t] > CHUNK
            dsel = dA if useA else dB
            doth = dB if useA else dA
            lsel = la if useA else lb
            loth = lb if useA else la
            members = np.where(tl == t)[0]
            excess = lsel[t] - CHUNK
            done = False
            for k in members[np.argsort(-dsel[nodes[members]])]:
                v = nodes[k]
                # find a partner tile+node: swapping v with u reduces t's
                # overfull half without overflowing anything else
                for t2 in np.argsort(lsel)[:8]:
                    if t2 == t:
                        continue
                    mem2 = np.where(tl == t2)[0]
                    for k2 in mem2[np.argsort(dsel[nodes[mem2]])[:4]]:
                        u = nodes[k2]
                        gain = dsel[v] - dsel[u]
                        if gain < excess * 0 + 1:
                            continue
                        nlsel_t2 = lsel[t2] - dsel[u] + dsel[v]
                        nloth_t2 = loth[t2] - doth[u] + doth[v]
                        nloth_t = loth[t] - doth[v] + doth[u]
                        if (nlsel_t2 <= CHUNK and nloth_t2 <= CHUNK
                                and nloth_t <= CHUNK):
                            tl[k], tl[k2] = t2, t
                            la[t] += dA[u] - dA[v]; lb[t] += dB[u] - dB[v]
                            la[t2] += dA[v] - dA[u]; lb[t2] += dB[v] - dB[u]
                            done = True
                            break
                    if done:
                        break
                if done:
                    break
            if not done:
                break
        tile_of[nodes] = tl
        # assign slots within tiles in order
        for t in range(NT):
            members = nodes[tl == t]
            slot_of[members] = np.arange(len(members))

    return core_of, tile_of, slot_of


def host_prep(x, edge_index, W1, b1, W2, b2):
    src = np.asarray(edge_index[0], dtype=np.int64)
    dst = np.asarray(edge_index[1], dtype=np.int64)

    core_of, tile_of, slot_of = _balance(src, dst)
    row_of = core_of * SHP + tile_of * BLK + slot_of  # new table row per node

    deg_full = np.bincount(dst, minlength=N_NODES).astype(np.float32) + 1.0

    # per-edge fields
    ec = core_of[dst]
    et = tile_of[dst]
    edl = slot_of[dst]
    erow = row_of[src]
    eh = (erow >= HALF).astype(np.int64)
    elrow = erow - eh * HALF

    # B[t, h]: blocks per (tile, half) = global max over cores (normally 8)
    counts = np.zeros((CORES, NT, 2), np.int64)
    np.add.at(counts, (ec, et, eh), 1)
    B = np.maximum(1, -(-counts.max(axis=0) // BLK))  # [NT, 2]
    nblocks = [int(B[:, h].sum()) for h in (0, 1)]
    nchunks = [-(-nblocks[h] // CHUNK_BLOCKS) for h in (0, 1)]
    startgb = np.zeros((NT, 2), np.int64)
    acc = [0, 0]
    for t in range(NT):
        for h in (0, 1):
            startgb[t, h] = acc[h]
            acc[h] += B[t, h]

    # per-core streams
    okey = (ec * NT + et) * 2 + eh
    eorder = np.lexsort((elrow, okey))
    okey_s = okey[eorder]
    lrow_s = elrow[eorder]
    dl_s = edl[eorder]
    bounds = np.searchsorted(okey_s, np.arange(CORES * NT * 2 + 1))

    data = []
    for i in range(CORES):
        planes, dls = [], []
        for h in (0, 1):
            parts_i, parts_d = [], []
            for t in range(NT):
                k = (i * NT + t) * 2 + h
                a, b = bounds[k], bounds[k + 1]
                r, d = lrow_s[a:b], dl_s[a:b]
                pad = int(B[t, h]) * BLK - (b - a)
                assert pad >= 0, f"bucket overflow core{i} t{t} h{h}"
                parts_i.append(np.concatenate(
                    [r, np.full(pad, ZROW, np.int64)]))
                parts_d.append(np.concatenate([d, np.zeros(pad, np.int64)]))
            si = np.concatenate(parts_i)
            sd = np.concatenate(parts_d)
            tail = nchunks[h] * CHUNK - len(si)
            si = np.concatenate([si, np.full(tail, ZROW, np.int64)])
            pl = si.reshape(-1, 16).T.astype(np.int16)
            planes.append(np.tile(pl, (8, 1)))
            dls.append(np.ascontiguousarray(
                sd.reshape(-1, BLK).T.astype(np.float32)))

        # deg plane [128, NT] and occupancy mask
        nodes_i = np.where(core_of == i)[0]
        degp = np.ones((BLK, NT), np.float32)
        maskp = np.zeros((BLK, NT), np.float32)
        degp[slot_of[nodes_i], tile_of[nodes_i]] = deg_full[nodes_i]
        maskp[slot_of[nodes_i], tile_of[nodes_i]] = 1.0

        # xT shard [96, 6272]
        xs = np.zeros((F0, SHP), np.float32)
        xs[:, tile_of[nodes_i] * BLK + slot_of[nodes_i]] = \
            np.asarray(x, np.float32)[nodes_i].T
        import ml_dtypes
        data.append(dict(
            xT=np.ascontiguousarray(xs.astype(ml_dtypes.bfloat16)),
            idx0=np.ascontiguousarray(planes[0]),
            idx1=np.ascontiguousarray(planes[1]),
            dl0=np.ascontiguousarray(dls[0]),
            dl1=np.ascontiguousarray(dls[1]),
            deg=np.ascontiguousarray(degp),
            mask=np.ascontiguousarray(maskp),
        ))

    import ml_dtypes
    consts = dict(
        W1=np.asarray(W1, np.float32).astype(ml_dtypes.bfloat16),
        W2=np.asarray(W2, np.float32),
        b1b=np.tile(np.asarray(b1, np.float32), (BLK, 1)),
        b2b=np.tile(np.asarray(b2, np.float32), (BLK, 1)),
        iota=np.tile(np.arange(BLK, dtype=np.float32), (BLK, 1)),
        ident=np.eye(BLK, dtype=np.float32),
    )
    meta = dict(B=B, nblocks=nblocks, nchunks=nchunks, startgb=startgb)
    perm = dict(core_of=core_of, tile_of=tile_of, slot_of=slot_of)
    return data, consts, meta, perm


def numpy_device_sim(data, consts, meta):
    """Replay the device algorithm in numpy (host-side validation)."""
    B, startgb, nblocks = meta["B"], meta["startgb"], meta["nblocks"]
    tables, dinvs = [], []
    for i in range(CORES):
        d = data[i]
        dinv = 1.0 / np.sqrt(d["deg"])
        dinvs.append(dinv)
        h = d["xT"].T @ consts["W1"]
        hs = h.reshape(NT, BLK, F1) * dinv.T[:, :, None]
        tables.append(hs.reshape(SHP, F1))
    table = np.concatenate(tables, 0)

    def layer(table, d):
        halves = [table[:HALF], table[HALF:]]
        agg = np.zeros((NT, F1, BLK), np.float32)
        for h in (0, 1):
            plane = d["idx0"] if h == 0 else d["idx1"]
            dl = d["dl0"] if h == 0 else d["dl1"]
            stream = plane[:16].T.reshape(-1)
            for t in range(NT):
                for b_ in range(int(B[t, h])):
                    gb = int(startgb[t, h]) + b_
                    rows = stream[gb * BLK:(gb + 1) * BLK].astype(np.int64)
                    G = halves[h][rows]
                    dloc = dl[:, gb].astype(np.int64)
                    S = np.zeros((BLK, BLK), np.float32)
                    S[np.arange(BLK), dloc] = 1.0
                    agg[t] += G.T @ S
        return agg

    full2 = []
    for i in range(CORES):
        d = data[i]
        agg = layer(table, d)
        dinv, mask = dinvs[i], d["mask"]
        tloc = tables[i].reshape(NT, BLK, F1)
        t2 = []
        for t in range(NT):
            a = agg[t].T + tloc[t]  # self term
            e = np.maximum(a * dinv[:, t:t + 1] + consts["b1b"], 0.0) \
                * (dinv[:, t:t + 1] * mask[:, t:t + 1])
            t2.append(e)
        full2.append(np.stack(t2).reshape(SHP, F1))
    table2 = np.concatenate(full2, 0)

    outs = []
    for i in range(CORES):
        d = data[i]
        agg = layer(table2, d)
        dinv = dinvs[i]
        t2loc = full2[i].reshape(NT, BLK, F1)
        o = np.zeros((NT, BLK, F2), np.float32)
        for t in range(NT):
            a = (agg[t].T + t2loc[t]) * dinv[:, t:t + 1]
            z = a @ consts["W2"] + consts["b2b"]
            m = z.max(1, keepdims=True)
            o[t] = z - m - np.log(np.exp(z - m).sum(1, keepdims=True))
        outs.append(o.reshape(SHP, F2))
    return np.stack(outs)


def assemble_output(outs, perm):
    res = np.zeros((N_NODES, F2), np.float32)
    rows = perm["tile_of"] * BLK + perm["slot_of"]
    for i in range(CORES):
        sel = perm["core_of"] == i
        res[np.where(sel)[0]] = outs[i][rows[sel]]
    return res


def build_nc(meta):
    import concourse.bacc as bacc
    import concourse.tile as tile
    import concourse.mybir as mybir

    dt = mybir.dt.float32
    Alu = mybir.AluOpType
    Act = mybir.ActivationFunctionType
    B, nblocks, nchunks, startgb = (
        meta["B"], meta["nblocks"], meta["nchunks"], meta["startgb"])

    nc = bacc.Bacc(None, target_bir_lowering=False)
    p_xT = nc.declare_dram_parameter("xT", [F0, SHP], mybir.dt.bfloat16,
                                     isOutput=False)
    p_idx = [nc.declare_dram_parameter(f"idx{h}", [128, nchunks[h] * (CHUNK // 16)],
                                       mybir.dt.int16, isOutput=False) for h in (0, 1)]
    p_dl = [nc.declare_dram_parameter(f"dl{h}", [128, nblocks[h]], dt, isOutput=False)
            for h in (0, 1)]
    p_deg = nc.declare_dram_parameter("deg", [128, NT], dt, isOutput=False)
    p_mask = nc.declare_dram_parameter("mask", [128, NT], dt, isOutput=False)
    p_W1 = nc.declare_dram_parameter("W1", [F0, F1], mybir.dt.bfloat16,
                                     isOutput=False)
    p_W2 = nc.declare_dram_parameter("W2", [F1, F2], dt, isOutput=False)
    p_b1 = nc.declare_dram_parameter("b1b", [128, F1], dt, isOutput=False)
    p_b2 = nc.declare_dram_parameter("b2b", [128, F2], dt, isOutput=False)
    p_iota = nc.declare_dram_parameter("iota", [128, 128], dt, isOutput=False)
    p_ident = nc.declare_dram_parameter("ident", [128, 128], dt, isOutput=False)
    p_out = nc.declare_dram_parameter("out", [128, NT * F2], dt, isOutput=True)

    cc_in = [nc.dram_tensor(f"cc_in{li}", [SHP, F1], dt) for li in (0, 1)]
    cc_out = [nc.dram_tensor(f"cc_out{li}", [CORES * SHP, F1], dt, addr_space="Shared")
              for li in (0, 1)]

    with tile.TileContext(nc) as tc:
        with (
            tc.tile_pool(name="cpool", bufs=1) as cpool,
            tc.tile_pool(name="spool", bufs=4) as spool,
            tc.tile_pool(name="stpool", bufs=10) as stpool,
            tc.tile_pool(name="wpool", bufs=4) as wpool,
            tc.tile_pool(name="ppool", bufs=3, space="PSUM") as ppool,
            tc.tile_pool(name="popool", bufs=2, space="PSUM") as popool,
        ):
            # ---- constants into SBUF
            xT = cpool.tile([F0, SHP], mybir.dt.bfloat16)
            for t in range(NT):
                nc.sync.dma_start(xT[:, BLK * t:BLK * (t + 1)],
                                  p_xT[:, BLK * t:BLK * (t + 1)])
            W1 = cpool.tile([F0, F1], mybir.dt.bfloat16)
            nc.sync.dma_start(W1[:], p_W1[:])
            W2 = cpool.tile([F1, F2], dt)
            nc.sync.dma_start(W2[:], p_W2[:])
            b1b = cpool.tile([128, F1], dt)
            nc.sync.dma_start(b1b[:], p_b1[:])
            b2b = cpool.tile([128, F2], dt)
            nc.sync.dma_start(b2b[:], p_b2[:])
            iota = cpool.tile([128, 128], dt)
            nc.sync.dma_start(iota[:], p_iota[:])
            ident = cpool.tile([128, 128], dt)
            nc.sync.dma_start(ident[:], p_ident[:])
            degt = cpool.tile([128, NT], dt)
            nc.sync.dma_start(degt[:], p_deg[:])
            maskt = cpool.tile([128, NT], dt)
            nc.sync.dma_start(maskt[:], p_mask[:])
            idx_sb = []
            dl_sb = []
            for h in (0, 1):
                isb = cpool.tile([128, nchunks[h] * (CHUNK // 16)], mybir.dt.int16,
                                 name=f"isb{h}")
                nc.sync.dma_start(isb[:], p_idx[h][:])
                idx_sb.append(isb)
                dsb = cpool.tile([128, nblocks[h]], dt, name=f"dsb{h}")
                nc.sync.dma_start(dsb[:], p_dl[h][:])
                dl_sb.append(dsb)

            recd = cpool.tile([128, NT], dt)
            nc.vector.reciprocal(recd[:], degt[:])
            dinv = cpool.tile([128, NT], dt)
            nc.scalar.activation(dinv[:], recd[:], Act.Sqrt)
            dinvm = cpool.tile([128, NT], dt)
            nc.vector.tensor_tensor(out=dinvm[:], in0=dinv[:], in1=maskt[:],
                                    op=Alu.mult)

            # ---- head: T1 shard = dinv * (x @ W1)
            Tsh = cpool.tile([128, NT * F1], dt)
            for t in range(NT):
                psh = ppool.tile([128, F1], dt, tag="agg1", name=f"psh{t}")
                nc.tensor.matmul(psh[:], xT[:, BLK * t:BLK * (t + 1)], W1[:],
                                 start=True, stop=True)
                nc.vector.tensor_scalar(
                    Tsh[:, F1 * t:F1 * (t + 1)], psh[:], dinv[:, t:t + 1], None,
                    Alu.mult)
                nc.sync.dma_start(cc_in[0][BLK * t:BLK * (t + 1), :],
                                  Tsh[:, F1 * t:F1 * (t + 1)])
            nc.gpsimd.collective_compute(
                "AllGather", Alu.bypass,
                ins=[cc_in[0].ap().opt()], outs=[cc_out[0].ap().opt()],
                replica_groups=[list(range(CORES))])

            def do_layer(li, table, self_tab, tail_fn):
                halves = [table[0:HALF, :], table[HALF:2 * HALF, :]]
                emitted = [0, 0]
                chunks = [{}, {}]

                def ensure_chunk(h, c):
                    while emitted[h] <= min(c + 4, nchunks[h] - 1):
                        ce = emitted[h]
                        st = stpool.tile([128, CHUNK_BLOCKS, F1], dt,
                                         tag=f"st{h}", name=f"st_l{li}_h{h}_c{ce}")
                        cols = CHUNK // 16
                        nc.gpsimd.dma_gather(
                            st[:], halves[h], idx_sb[h][:, ce * cols:(ce + 1) * cols],
                            CHUNK, CHUNK, F1)
                        chunks[h][ce] = st
                        emitted[h] += 1
                    return chunks[h][c]

                for t in range(NT):
                    nb = int(B[t, 0] + B[t, 1])
                    pagg = ppool.tile([128, F1], dt, tag="agg1", name=f"pg{li}_{t}")
                    k = 0
                    for h in (0, 1):
                        for b in range(int(B[t, h])):
                            gb = int(startgb[t, h]) + b
                            c, slot = gb // CHUNK_BLOCKS, gb % CHUNK_BLOCKS
                            st = ensure_chunk(h, c)
                            S = spool.tile([128, 128], dt, tag="S",
                                           name=f"S{li}_{t}_{h}_{b}")
                            nc.vector.tensor_scalar(
                                S[:], iota[:], dl_sb[h][:, gb:gb + 1], None,
                                Alu.is_equal)
                            nc.tensor.matmul(pagg[:], S[:], st[:, slot, :],
                                             start=(k == 0), stop=(k == nb - 1))
                            k += 1
                    tail_fn(t, pagg)

            # ---- layer 1
            T2sh = cpool.tile([128, NT * F1], dt)

            def tail1(t, pagg):
                e0 = wpool.tile([128, F1], dt, tag="e0", name=f"e0_{t}")
                nc.vector.tensor_tensor(out=e0[:], in0=pagg[:],
                                        in1=Tsh[:, F1 * t:F1 * (t + 1)], op=Alu.add)
                e1 = wpool.tile([128, F1], dt, tag="e1", name=f"e1_{t}")
                nc.vector.tensor_scalar(e1[:], e0[:], dinv[:, t:t + 1], None,
                                        Alu.mult)
                e2 = wpool.tile([128, F1], dt, tag="e2", name=f"e2_{t}")
                nc.vector.tensor_tensor(out=e2[:], in0=e1[:], in1=b1b[:], op=Alu.add)
                nc.vector.tensor_scalar(
                    T2sh[:, F1 * t:F1 * (t + 1)], e2[:], 0.0, dinvm[:, t:t + 1],
                    Alu.max, Alu.mult)
                nc.sync.dma_start(cc_in[1][BLK * t:BLK * (t + 1), :],
                                  T2sh[:, F1 * t:F1 * (t + 1)])

            do_layer(0, cc_out[0], None, tail1)
            nc.gpsimd.collective_compute(
                "AllGather", Alu.bypass,
                ins=[cc_in[1].ap().opt()], outs=[cc_out[1].ap().opt()],
                replica_groups=[list(range(CORES))])

            # ---- layer 2

            def tail2(t, pagg):
                a0 = wpool.tile([128, F1], dt, tag="a0", name=f"a0_{t}")
                nc.vector.tensor_tensor(out=a0[:], in0=pagg[:],
                                        in1=T2sh[:, F1 * t:F1 * (t + 1)],
                                        op=Alu.add)
                pt = popool.tile([F1, BLK], dt, tag="pt", name=f"pt_{t}")
                nc.tensor.matmul(pt[:], a0[:], ident[:], start=True, stop=True)
                aggS = wpool.tile([F1, BLK], dt, tag="aggS", name=f"as_{t}")
                nc.vector.tensor_copy(aggS[:], pt[:])
                po = popool.tile([128, F2], dt, tag="po", name=f"po_{t}")
                nc.tensor.matmul(po[:], aggS[:], W2[:], start=True, stop=True)
                e3 = wpool.tile([128, F2], dt, tag="e3", name=f"e3_{t}")
                nc.vector.tensor_scalar(e3[:], po[:], dinv[:, t:t + 1], None,
                                        Alu.mult)
                e4 = wpool.tile([128, F2], dt, tag="e4", name=f"e4_{t}")
                nc.vector.tensor_tensor(out=e4[:], in0=e3[:], in1=b2b[:], op=Alu.add)
                m = wpool.tile([128, 1], dt, tag="m", name=f"m_{t}")
                nc.vector.tensor_reduce(m[:], e4[:], axis=mybir.AxisListType.X,
                                        op=Alu.max)
                nm = wpool.tile([128, 1], dt, tag="nm", name=f"nm_{t}")
                nc.vector.tensor_scalar(nm[:], m[:], -1.0, None, Alu.mult)
                ex = wpool.tile([128, F2], dt, tag="ex", name=f"ex_{t}")
                nc.scalar.activation(ex[:], e4[:], Act.Exp, bias=nm[:, 0:1])
                sm = wpool.tile([128, 1], dt, tag="sm", name=f"sm_{t}")
                nc.vector.tensor_reduce(sm[:], ex[:], axis=mybir.AxisListType.X,
                                        op=Alu.add)
                lg = wpool.tile([128, 1], dt, tag="lg", name=f"lg_{t}")
                nc.scalar.activation(lg[:], sm[:], Act.Ln)
                fo = wpool.tile([128, F2], dt, tag="fo", name=f"fo_{t}")
                nc.vector.tensor_scalar(
                    fo[:], e4[:], m[:, 0:1], lg[:, 0:1],
                    Alu.subtract, Alu.subtract)
                nc.sync.dma_start(p_out[:, F2 * t:F2 * (t + 1)], fo[:])

            def tail2_scaled(t, pagg):
                # pagg already includes the transposed self term; note the
                # dinv scale applies AFTER adding self (both are pre-scaled
                # table rows), matching tail2's e3 step.
                tail2(t, pagg)

            do_layer(1, cc_out[1], T2sh, tail2_scaled)

    nc.finalize()
    return nc


LAST_EXEC_NS = None


def kernel(x, edge_index, W1, b1, W2, b2):
    from concourse.bass_utils import run_bass_kernel_spmd

    x = np.asarray(x, np.float32)
    data, consts, meta, perm = host_prep(x, np.asarray(edge_index), W1, b1, W2, b2)
    nc = build_nc(meta)
    in_maps = []
    for i in range(CORES):
        m = dict(data[i])
        m.update({k: np.ascontiguousarray(v) for k, v in consts.items()})
        in_maps.append(m)
    import os as _os
    trace = bool(int(_os.environ.get("GCN_TRACE", "0")))
    res = run_bass_kernel_spmd(nc, in_maps, core_ids=list(range(CORES)), trace=trace)
    global LAST_EXEC_NS
    LAST_EXEC_NS = res.exec_time_ns
    outs = []
    for i in range(CORES):
        o = res.results[i]["out"]  # [128, NT*F2]
        outs.append(o.reshape(128, NT, F2).transpose(1, 0, 2).reshape(SHP, F2))
    return assemble_output(np.stack(outs), perm)


if __name__ == "__main__":
    import reference
    inputs = {k: np.asarray(v) for k, v in reference.setup_inputs().items()}
    expected = np.asarray(reference.reference(**{k: v for k, v in inputs.items()}))
    data, consts, meta, perm = host_prep(**inputs)
    print("B max:", meta["B"].max(), "nblocks:", meta["nblocks"],
          "nchunks:", meta["nchunks"])
    outs = numpy_device_sim(data, consts, meta)
    got = assemble_output(outs, perm)
    err = np.abs(got - expected)
    rel = err.max() / np.abs(expected).max()
    print(f"numpy-sim max abs err {err.max():.3e}  rel {rel:.3e}")


# revision 9
# speedup vs baseline: 1.1979x; 1.1979x over previous
"""GCN (2-layer) Trainium2 kernel over 8 NeuronCores.

Strategy:
- GCN is node-permutation-equivariant: the host renumbers nodes so that
  (a) each core owns 6250 nodes (padded shard 6272 = 49 tiles x 128),
  (b) in-edge counts are balanced so that every (dst-tile, src-half) bucket
      holds <= 1024 edges -> exactly 8 gather-blocks of 128, no max-over-core
      padding.  This minimizes the GPSIMD SWDGE descriptor-generation time,
      which is the hard serial bottleneck (~8ns/edge on the Pool engine).
- h1 = (x @ W1) scaled by dinv (deg^-1/2) computed shard-local -> AllGather
  to a full 50176-row table in each core's DRAM.
- Scatter-add aggregation out[d] += table[src] over REAL edges only is done
  per dst-core: rows fetched with gpsimd.dma_gather (int16 idx over two
  25088-row halves); segmented sum per 128-dst tile via TensorE matmul with
  a DVE-built one-hot selector.  The self-loop term is added locally from
  the resident shard (no gather traffic).
- Layer 2 aggregates the dinv-scaled relu table (64 feats), then W2 + bias +
  log_softmax on-chip.
The edge structure is baked into the program at build time (SPMD; identical
program on all 8 cores, per-core data differs, padded to a common shape).
"""

import numpy as np

N_NODES = 50000
CORES = 8
SH = 6250          # owned nodes per core
SHP = 6272         # padded shard rows (49*128)
NT = 49            # dst tiles per core
HALF = SHP * 4     # 25088 table rows per half (cores 0-3 | 4-7)
F0, F1, F2 = 96, 64, 16
BLK = 128
CHUNK_BLOCKS = 8   # 1024 idx per dma_gather (single_packet limit)
CHUNK = BLK * CHUNK_BLOCKS
ZROW = 25087       # half-local row of a guaranteed-zero table row (both halves)


# --------------------------------------------------------------------------
# host-side balancing: nodes -> (core, tile, slot)
# --------------------------------------------------------------------------

def _balance(src, dst):
    """Assign nodes to cores/tiles/slots, balancing real-edge in-degrees so
    that each (core, tile, src-half) bucket holds close to 1024 edges."""
    N = N_NODES
    deg = np.bincount(dst, minlength=N).astype(np.int64)

    # ---- phase 1: core assignment, balance total in-degree, cap 6250 ----
    order = np.argsort(-deg, kind="stable")
    core_of = np.empty(N, np.int64)
    load = np.zeros(CORES, np.int64)
    cnt = np.zeros(CORES, np.int64)
    for v in order:
        c = -1
        best = None
        for k in range(CORES):
            if cnt[k] < SH and (best is None or load[k] < best):
                best = load[k]
                c = k
        core_of[v] = c
        load[c] += deg[v]
        cnt[c] += 1

    # ---- phase 2: balance A/B half split of each core's in-edges --------
    # O[u, c] = # out-edges of u landing in core c
    O = np.zeros((N, CORES), np.int32)
    np.add.at(O, (src, core_of[dst]), 1)
    in_tot = load.copy()  # in-edges per core

    def half_excess():
        # E(c, A) - in(c)/2 for each core c (A = src core < 4)
        srcA = core_of[src] < 4
        EA = np.bincount(core_of[dst][srcA], minlength=CORES)
        return EA - in_tot / 2.0

    delta = half_excess()
    isA = core_of < 4
    for _ in range(400):
        w = int(np.argmax(np.abs(delta)))
        if abs(delta[w]) <= 16:
            break
        # delta[w] > 0: too many in-edges of w from half A -> move a node
        # with many out-edges-into-w from A to B (swap with equal-degree).
        if delta[w] > 0:
            cand1 = np.where(isA)[0]
            cand2mask = ~isA
        else:
            cand1 = np.where(~isA)[0]
            cand2mask = isA
        u1 = cand1[np.argmax(O[cand1, w])]
        d1 = deg[u1]
        cand2 = np.where(cand2mask & (deg == d1))[0]
        if len(cand2) == 0:
            break
        u2 = cand2[np.argmin(O[cand2, w])]
        # swap cores of u1, u2 (equal in-degree keeps in_tot intact)
        c1, c2 = core_of[u1], core_of[u2]
        core_of[u1], core_of[u2] = c2, c1
        isA[u1], isA[u2] = c2 < 4, c1 < 4
        # update delta incrementally: u1 moved A->B (or B->A), u2 opposite
        s1 = 1.0 if c1 < 4 else -1.0  # u1 leaves half(c1)
        delta = delta - s1 * O[u1] + s1 * O[u2]

    # ---- phase 3: per-core tile packing, 2D (dA, dB) <= (1024, 1024) ----
    srcA = core_of[src] < 4
    dA = np.bincount(dst[srcA], minlength=N).astype(np.int64)
    dB = deg - dA

    tile_of = np.empty(N, np.int64)
    slot_of = np.empty(N, np.int64)
    for c in range(CORES):
        nodes = np.where(core_of == c)[0]
        nodes = nodes[np.argsort(-(dA[nodes] + dB[nodes]), kind="stable")]
        cap = np.full(NT, BLK, np.int64)
        if c in (3, 7):
            cap[NT - 1] = BLK - 1  # reserve the ZROW slot
        la = np.zeros(NT, np.int64)
        lb = np.zeros(NT, np.int64)
        nc_ = np.zeros(NT, np.int64)
        tl = np.empty(len(nodes), np.int64)
        for i, v in enumerate(nodes):
            score = np.maximum(la + dA[v], lb + dB[v]).astype(np.float64)
            score[nc_ >= cap] = np.inf
            t = int(np.argmin(score))
            tl[i] = t
            la[t] += dA[v]
            lb[t] += dB[v]
            nc_[t] += 1
        # repair: move nodes out of overfull bins (either half > 1024)
        for _ in range(600):
            over = np.where((la > CHUNK) | (lb > CHUNK))[0]
            if len(over) == 0:
                break
            t = int(over[0])
            halfsel = la if la[t] > CHUNK else lb
            dsel = dA if la[t] > CHUNK else dB
            members = np.where(tl == t)[0]
            excess = halfsel[t] - CHUNK
            diffs = dsel[nodes[members]]
            k = members[np.argmin(np.abs(diffs - excess))]
            v = nodes[k]
            score = np.maximum(la + dA[v], lb + dB[v]).astype(np.float64)
            score[nc_ >= cap] = np.inf
            score[t] = np.inf
            t2 = int(np.argmin(score))
            tl[k] = t2
            la[t] -= dA[v]; lb[t] -= dB[v]; nc_[t] -= 1
            la[t2] += dA[v]; lb[t2] += dB[v]; nc_[t2] += 1
        # swap repair: exchange nodes between overfull/underfull tiles
        for _ in range(400):
            over = np.where((la > CHUNK) | (lb > CHUNK))[0]
            if len(over) == 0:
                break
            t = int(over[0])
            useA = la[t] > CHUNK
            dsel = dA if useA else dB
            doth = dB if useA else dA
            lsel = la if useA else lb
            loth = lb if useA else la
            members = np.where(tl == t)[0]
            excess = lsel[t] - CHUNK
            done = False
            for k in members[np.argsort(-dsel[nodes[members]])]:
                v = nodes[k]
                # find a partner tile+node: swapping v with u reduces t's
                # overfull half without overflowing anything else
                for t2 in np.argsort(lsel)[:8]:
                    if t2 == t:
                        continue
                    mem2 = np.where(tl == t2)[0]
                    for k2 in mem2[np.argsort(dsel[nodes[mem2]])[:4]]:
                        u = nodes[k2]
                        gain = dsel[v] - dsel[u]
                        if gain < excess * 0 + 1:
                            continue
                        nlsel_t2 = lsel[t2] - dsel[u] + dsel[v]
                        nloth_t2 = loth[t2] - doth[u] + doth[v]
                        nloth_t = loth[t] - doth[v] + doth[u]
                        if (nlsel_t2 <= CHUNK and nloth_t2 <= CHUNK
                                and nloth_t <= CHUNK):
                            tl[k], tl[k2] = t2, t
                            la[t] += dA[u] - dA[v]; lb[t] += dB[u] - dB[v]
                            la[t2] += dA[v] - dA[u]; lb[t2] += dB[v] - dB[u]
                            done = True
                            break
                    if done:
                        break
                if done:
                    break
            if not done:
                break
        tile_of[nodes] = tl
        # assign slots within tiles in order
        for t in range(NT):
            members = nodes[tl == t]
            slot_of[members] = np.arange(len(members))

    return core_of, tile_of, slot_of


def host_prep(x, edge_index, W1, b1, W2, b2):
    src = np.asarray(edge_index[0], dtype=np.int64)
    dst = np.asarray(edge_index[1], dtype=np.int64)

    core_of, tile_of, slot_of = _balance(src, dst)
    row_of = core_of * SHP + tile_of * BLK + slot_of  # new table row per node

    deg_full = np.bincount(dst, minlength=N_NODES).astype(np.float32) + 1.0

    # per-edge fields
    ec = core_of[dst]
    et = tile_of[dst]
    edl = slot_of[dst]
    erow = row_of[src]
    eh = (erow >= HALF).astype(np.int64)
    elrow = erow - eh * HALF

    # B[t, h]: blocks per (tile, half) = global max over cores (normally 8)
    counts = np.zeros((CORES, NT, 2), np.int64)
    np.add.at(counts, (ec, et, eh), 1)
    B = np.maximum(1, -(-counts.max(axis=0) // BLK))  # [NT, 2]
    nblocks = [int(B[:, h].sum()) for h in (0, 1)]
    nchunks = [-(-nblocks[h] // CHUNK_BLOCKS) for h in (0, 1)]
    startgb = np.zeros((NT, 2), np.int64)
    acc = [0, 0]
    for t in range(NT):
        for h in (0, 1):
            startgb[t, h] = acc[h]
            acc[h] += B[t, h]

    # per-core streams
    okey = (ec * NT + et) * 2 + eh
    eorder = np.lexsort((elrow, okey))
    okey_s = okey[eorder]
    lrow_s = elrow[eorder]
    dl_s = edl[eorder]
    bounds = np.searchsorted(okey_s, np.arange(CORES * NT * 2 + 1))

    data = []
    for i in range(CORES):
        planes, dls = [], []
        for h in (0, 1):
            parts_i, parts_d = [], []
            for t in range(NT):
                k = (i * NT + t) * 2 + h
                a, b = bounds[k], bounds[k + 1]
                r, d = lrow_s[a:b], dl_s[a:b]
                pad = int(B[t, h]) * BLK - (b - a)
                assert pad >= 0, f"bucket overflow core{i} t{t} h{h}"
                parts_i.append(np.concatenate(
                    [r, np.full(pad, ZROW, np.int64)]))
                parts_d.append(np.concatenate([d, np.zeros(pad, np.int64)]))
            si = np.concatenate(parts_i)
            sd = np.concatenate(parts_d)
            tail = nchunks[h] * CHUNK - len(si)
            si = np.concatenate([si, np.full(tail, ZROW, np.int64)])
            pl = si.reshape(-1, 16).T.astype(np.int16)
            planes.append(np.tile(pl, (8, 1)))
            dls.append(np.ascontiguousarray(
                sd.reshape(-1, BLK).T.astype(np.float32)))

        # deg plane [128, NT] and occupancy mask
        nodes_i = np.where(core_of == i)[0]
        degp = np.ones((BLK, NT), np.float32)
        maskp = np.zeros((BLK, NT), np.float32)
        degp[slot_of[nodes_i], tile_of[nodes_i]] = deg_full[nodes_i]
        maskp[slot_of[nodes_i], tile_of[nodes_i]] = 1.0

        # xT shard [96, 6272]
        xs = np.zeros((F0, SHP), np.float32)
        xs[:, tile_of[nodes_i] * BLK + slot_of[nodes_i]] = \
            np.asarray(x, np.float32)[nodes_i].T
        import ml_dtypes
        data.append(dict(
            xT=np.ascontiguousarray(xs.astype(ml_dtypes.bfloat16)),
            idx0=np.ascontiguousarray(planes[0]),
            idx1=np.ascontiguousarray(planes[1]),
            dl0=np.ascontiguousarray(dls[0]),
            dl1=np.ascontiguousarray(dls[1]),
            deg=np.ascontiguousarray(degp),
            mask=np.ascontiguousarray(maskp),
        ))

    import ml_dtypes
    consts = dict(
        W1=np.asarray(W1, np.float32).astype(ml_dtypes.bfloat16),
        W2=np.asarray(W2, np.float32),
        b1b=np.tile(np.asarray(b1, np.float32), (BLK, 1)),
        b2b=np.tile(np.asarray(b2, np.float32), (BLK, 1)),
        iota=np.tile(np.arange(BLK, dtype=np.float32), (BLK, 1)),
        ident=np.eye(BLK, dtype=np.float32),
    )
    meta = dict(B=B, nblocks=nblocks, nchunks=nchunks, startgb=startgb)
    perm = dict(core_of=core_of, tile_of=tile_of, slot_of=slot_of)
    return data, consts, meta, perm


def numpy_device_sim(data, consts, meta):
    """Replay the device algorithm in numpy (host-side validation)."""
    B, startgb, nblocks = meta["B"], meta["startgb"], meta["nblocks"]
    tables, dinvs = [], []
    for i in range(CORES):
        d = data[i]
        dinv = 1.0 / np.sqrt(d["deg"])
        dinvs.append(dinv)
        h = d["xT"].T @ consts["W1"]
        hs = h.reshape(NT, BLK, F1) * dinv.T[:, :, None]
        tables.append(hs.reshape(SHP, F1))
    table = np.concatenate(tables, 0)

    def layer(table, d):
        halves = [table[:HALF], table[HALF:]]
        agg = np.zeros((NT, F1, BLK), np.float32)
        for h in (0, 1):
            plane = d["idx0"] if h == 0 else d["idx1"]
            dl = d["dl0"] if h == 0 else d["dl1"]
            stream = plane[:16].T.reshape(-1)
            for t in range(NT):
                for b_ in range(int(B[t, h])):
                    gb = int(startgb[t, h]) + b_
                    rows = stream[gb * BLK:(gb + 1) * BLK].astype(np.int64)
                    G = halves[h][rows]
                    dloc = dl[:, gb].astype(np.int64)
                    S = np.zeros((BLK, BLK), np.float32)
                    S[np.arange(BLK), dloc] = 1.0
                    agg[t] += G.T @ S
        return agg

    full2 = []
    for i in range(CORES):
        d = data[i]
        agg = layer(table, d)
        dinv, mask = dinvs[i], d["mask"]
        tloc = tables[i].reshape(NT, BLK, F1)
        t2 = []
        for t in range(NT):
            a = agg[t].T + tloc[t]  # self term
            e = np.maximum(a * dinv[:, t:t + 1] + consts["b1b"], 0.0) \
                * (dinv[:, t:t + 1] * mask[:, t:t + 1])
            t2.append(e)
        full2.append(np.stack(t2).reshape(SHP, F1))
    table2 = np.concatenate(full2, 0)

    outs = []
    for i in range(CORES):
        d = data[i]
        agg = layer(table2, d)
        dinv = dinvs[i]
        t2loc = full2[i].reshape(NT, BLK, F1)
        o = np.zeros((NT, BLK, F2), np.float32)
        for t in range(NT):
            a = (agg[t].T + t2loc[t]) * dinv[:, t:t + 1]
            z = a @ consts["W2"] + consts["b2b"]
            m = z.max(1, keepdims=True)
            o[t] = z - m - np.log(np.exp(z - m).sum(1, keepdims=True))
        outs.append(o.reshape(SHP, F2))
    return np.stack(outs)


def assemble_output(outs, perm):
    res = np.zeros((N_NODES, F2), np.float32)
    rows = perm["tile_of"] * BLK + perm["slot_of"]
    for i in range(CORES):
        sel = perm["core_of"] == i
        res[np.where(sel)[0]] = outs[i][rows[sel]]
    return res


def build_nc(meta):
    import concourse.bacc as bacc
    import concourse.tile as tile
    import concourse.mybir as mybir

    dt = mybir.dt.float32
    Alu = mybir.AluOpType
    Act = mybir.ActivationFunctionType
    B, nblocks, nchunks, startgb = (
        meta["B"], meta["nblocks"], meta["nchunks"], meta["startgb"])

    nc = bacc.Bacc(None, target_bir_lowering=False)
    p_xT = nc.declare_dram_parameter("xT", [F0, SHP], mybir.dt.bfloat16,
                                     isOutput=False)
    p_idx = [nc.declare_dram_parameter(f"idx{h}", [128, nchunks[h] * (CHUNK // 16)],
                                       mybir.dt.int16, isOutput=False) for h in (0, 1)]
    p_dl = [nc.declare_dram_parameter(f"dl{h}", [128, nblocks[h]], dt, isOutput=False)
            for h in (0, 1)]
    p_deg = nc.declare_dram_parameter("deg", [128, NT], dt, isOutput=False)
    p_mask = nc.declare_dram_parameter("mask", [128, NT], dt, isOutput=False)
    p_W1 = nc.declare_dram_parameter("W1", [F0, F1], mybir.dt.bfloat16,
                                     isOutput=False)
    p_W2 = nc.declare_dram_parameter("W2", [F1, F2], dt, isOutput=False)
    p_b1 = nc.declare_dram_parameter("b1b", [128, F1], dt, isOutput=False)
    p_b2 = nc.declare_dram_parameter("b2b", [128, F2], dt, isOutput=False)
    p_iota = nc.declare_dram_parameter("iota", [128, 128], dt, isOutput=False)
    p_ident = nc.declare_dram_parameter("ident", [128, 128], dt, isOutput=False)
    p_out = nc.declare_dram_parameter("out", [128, NT * F2], dt, isOutput=True)

    cc_in = [nc.dram_tensor(f"cc_in{li}", [SHP, F1], dt) for li in (0, 1)]
    cc_out = [nc.dram_tensor(f"cc_out{li}", [CORES * SHP, F1], dt, addr_space="Shared")
              for li in (0, 1)]

    with tile.TileContext(nc) as tc:
        with (
            tc.tile_pool(name="cpool", bufs=1) as cpool,
            tc.tile_pool(name="spool", bufs=4) as spool,
            tc.tile_pool(name="stpool", bufs=8) as stpool,
            tc.tile_pool(name="wpool", bufs=4) as wpool,
            tc.tile_pool(name="ppool", bufs=3, space="PSUM") as ppool,
            tc.tile_pool(name="popool", bufs=2, space="PSUM") as popool,
        ):
            # ---- constants into SBUF
            xT = cpool.tile([F0, SHP], mybir.dt.bfloat16)
            nc.sync.dma_start(xT[:], p_xT[:])
            W1 = cpool.tile([F0, F1], mybir.dt.bfloat16)
            nc.sync.dma_start(W1[:], p_W1[:])
            W2 = cpool.tile([F1, F2], dt)
            nc.sync.dma_start(W2[:], p_W2[:])
            b1b = cpool.tile([128, F1], dt)
            nc.sync.dma_start(b1b[:], p_b1[:])
            b2b = cpool.tile([128, F2], dt)
            nc.sync.dma_start(b2b[:], p_b2[:])
            iota = cpool.tile([128, 128], dt)
            nc.sync.dma_start(iota[:], p_iota[:])
            ident = cpool.tile([128, 128], dt)
            nc.sync.dma_start(ident[:], p_ident[:])
            degt = cpool.tile([128, NT], dt)
            nc.sync.dma_start(degt[:], p_deg[:])
            maskt = cpool.tile([128, NT], dt)
            nc.sync.dma_start(maskt[:], p_mask[:])
            idx_sb = []
            dl_sb = []
            for h in (0, 1):
                isb = cpool.tile([128, nchunks[h] * (CHUNK // 16)], mybir.dt.int16,
                                 name=f"isb{h}")
                nc.sync.dma_start(isb[:], p_idx[h][:])
                idx_sb.append(isb)
                dsb = cpool.tile([128, nblocks[h]], dt, name=f"dsb{h}")
                nc.sync.dma_start(dsb[:], p_dl[h][:])
                dl_sb.append(dsb)

            recd = cpool.tile([128, NT], dt)
            nc.vector.reciprocal(recd[:], degt[:])
            dinv = cpool.tile([128, NT], dt)
            nc.scalar.activation(dinv[:], recd[:], Act.Sqrt)
            dinvm = cpool.tile([128, NT], dt)
            nc.vector.tensor_tensor(out=dinvm[:], in0=dinv[:], in1=maskt[:],
                                    op=Alu.mult)

            # ---- head: T1 shard = dinv * (x @ W1)
            Tsh = cpool.tile([128, NT * F1], dt)
            for t in range(NT):
                psh = ppool.tile([128, F1], dt, tag="agg1", name=f"psh{t}")
                nc.tensor.matmul(psh[:], xT[:, BLK * t:BLK * (t + 1)], W1[:],
                                 start=True, stop=True)
                nc.vector.tensor_scalar(
                    Tsh[:, F1 * t:F1 * (t + 1)], psh[:], dinv[:, t:t + 1], None,
                    Alu.mult)
                nc.sync.dma_start(cc_in[0][BLK * t:BLK * (t + 1), :],
                                  Tsh[:, F1 * t:F1 * (t + 1)])
            nc.gpsimd.collective_compute(
                "AllGather", Alu.bypass,
                ins=[cc_in[0].ap().opt()], outs=[cc_out[0].ap().opt()],
                replica_groups=[list(range(CORES))])

            def do_layer(li, table, self_tab, tail_fn):
                halves = [table[0:HALF, :], table[HALF:2 * HALF, :]]
                emitted = [0, 0]
                chunks = [{}, {}]

                def ensure_chunk(h, c):
                    while emitted[h] <= min(c + 3, nchunks[h] - 1):
                        ce = emitted[h]
                        st = stpool.tile([128, CHUNK_BLOCKS, F1], dt,
                                         tag=f"st{h}", name=f"st_l{li}_h{h}_c{ce}")
                        cols = CHUNK // 16
                        nc.gpsimd.dma_gather(
                            st[:], halves[h], idx_sb[h][:, ce * cols:(ce + 1) * cols],
                            CHUNK, CHUNK, F1)
                        chunks[h][ce] = st
                        emitted[h] += 1
                    return chunks[h][c]

                for t in range(NT):
                    nb = int(B[t, 0] + B[t, 1])
                    pagg = ppool.tile([128, F1], dt, tag="agg1", name=f"pg{li}_{t}")
                    k = 0
                    for h in (0, 1):
                        for b in range(int(B[t, h])):
                            gb = int(startgb[t, h]) + b
                            c, slot = gb // CHUNK_BLOCKS, gb % CHUNK_BLOCKS
                            st = ensure_chunk(h, c)
                            S = spool.tile([128, 128], dt, tag="S",
                                           name=f"S{li}_{t}_{h}_{b}")
                            nc.vector.tensor_scalar(
                                S[:], iota[:], dl_sb[h][:, gb:gb + 1], None,
                                Alu.is_equal)
                            nc.tensor.matmul(pagg[:], S[:], st[:, slot, :],
                                             start=(k == 0), stop=(k == nb - 1))
                            k += 1
                    tail_fn(t, pagg)

            # ---- layer 1
            T2sh = cpool.tile([128, NT * F1], dt)

            def tail1(t, pagg):
                e0 = wpool.tile([128, F1], dt, tag="e0", name=f"e0_{t}")
                nc.vector.tensor_tensor(out=e0[:], in0=pagg[:],
                                        in1=Tsh[:, F1 * t:F1 * (t + 1)], op=Alu.add)
                e1 = wpool.tile([128, F1], dt, tag="e1", name=f"e1_{t}")
                nc.vector.tensor_scalar(e1[:], e0[:], dinv[:, t:t + 1], None,
                                        Alu.mult)
                e2 = wpool.tile([128, F1], dt, tag="e2", name=f"e2_{t}")
                nc.vector.tensor_tensor(out=e2[:], in0=e1[:], in1=b1b[:], op=Alu.add)
                nc.vector.tensor_scalar(
                    T2sh[:, F1 * t:F1 * (t + 1)], e2[:], 0.0, dinvm[:, t:t + 1],
                    Alu.max, Alu.mult)
                nc.sync.dma_start(cc_in[1][BLK * t:BLK * (t + 1), :],
                                  T2sh[:, F1 * t:F1 * (t + 1)])

            do_layer(0, cc_out[0], None, tail1)
            nc.gpsimd.collective_compute(
                "AllGather", Alu.bypass,
                ins=[cc_in[1].ap().opt()], outs=[cc_out[1].ap().opt()],
                replica_groups=[list(range(CORES))])

            # ---- layer 2

            def tail2(t, pagg):
                a0 = wpool.tile([128, F1], dt, tag="a0", name=f"a0_{t}")
                nc.vector.tensor_tensor(out=a0[:], in0=pagg[:],
                                        in1=T2sh[:, F1 * t:F1 * (t + 1)],
                                        op=Alu.add)
                pt = popool.tile([F1, BLK], dt, tag="pt", name=f"pt_{t}")
                nc.tensor.matmul(pt[:], a0[:], ident[:], start=True, stop=True)
                aggS = wpool.tile([F1, BLK], dt, tag="aggS", name=f"as_{t}")
                nc.vector.tensor_copy(aggS[:], pt[:])
                po = popool.tile([128, F2], dt, tag="po", name=f"po_{t}")
                nc.tensor.matmul(po[:], aggS[:], W2[:], start=True, stop=True)
                e3 = wpool.tile([128, F2], dt, tag="e3", name=f"e3_{t}")
                nc.vector.tensor_scalar(e3[:], po[:], dinv[:, t:t + 1], None,
                                        Alu.mult)
                e4 = wpool.tile([128, F2], dt, tag="e4", name=f"e4_{t}")
                nc.vector.tensor_tensor(out=e4[:], in0=e3[:], in1=b2b[:], op=Alu.add)
                m = wpool.tile([128, 1], dt, tag="m", name=f"m_{t}")
                nc.vector.tensor_reduce(m[:], e4[:], axis=mybir.AxisListType.X,
                                        op=Alu.max)
                nm = wpool.tile([128, 1], dt, tag="nm", name=f"nm_{t}")
                nc.vector.tensor_scalar(nm[:], m[:], -1.0, None, Alu.mult)
                ex = wpool.tile([128, F2], dt, tag="ex", name=f"ex_{t}")
                nc.scalar.activation(ex[:], e4[:], Act.Exp, bias=nm[:, 0:1])
                sm = wpool.tile([128, 1], dt, tag="sm", name=f"sm_{t}")
                nc.vector.tensor_reduce(sm[:], ex[:], axis=mybir.AxisListType.X,
                                        op=Alu.add)
                lg = wpool.tile([128, 1], dt, tag="lg", name=f"lg_{t}")
                nc.scalar.activation(lg[:], sm[:], Act.Ln)
                fo = wpool.tile([128, F2], dt, tag="fo", name=f"fo_{t}")
                nc.vector.tensor_scalar(
                    fo[:], e4[:], m[:, 0:1], lg[:, 0:1],
                    Alu.subtract, Alu.subtract)
                nc.sync.dma_start(p_out[:, F2 * t:F2 * (t + 1)], fo[:])

            def tail2_scaled(t, pagg):
                # pagg already includes the transposed self term; note the
                # dinv scale applies AFTER adding self (both are pre-scaled
                # table rows), matching tail2's e3 step.
                tail2(t, pagg)

            do_layer(1, cc_out[1], T2sh, tail2_scaled)

    nc.finalize()
    return nc


LAST_EXEC_NS = None


def kernel(x, edge_index, W1, b1, W2, b2):
    from concourse.bass_utils import run_bass_kernel_spmd

    x = np.asarray(x, np.float32)
    data, consts, meta, perm = host_prep(x, np.asarray(edge_index), W1, b1, W2, b2)
    nc = build_nc(meta)
    in_maps = []
    for i in range(CORES):
        m = dict(data[i])
        m.update({k: np.ascontiguousarray(v) for k, v in consts.items()})
        in_maps.append(m)
    import os as _os
    trace = bool(int(_os.environ.get("GCN_TRACE", "0")))
    res = run_bass_kernel_spmd(nc, in_maps, core_ids=list(range(CORES)), trace=trace)
    global LAST_EXEC_NS
    LAST_EXEC_NS = res.exec_time_ns
    outs = []
    for i in range(CORES):
        o = res.results[i]["out"]  # [128, NT*F2]
        outs.append(o.reshape(128, NT, F2).transpose(1, 0, 2).reshape(SHP, F2))
    return assemble_output(np.stack(outs), perm)


if __name__ == "__main__":
    import reference
    inputs = {k: np.asarray(v) for k, v in reference.setup_inputs().items()}
    expected = np.asarray(reference.reference(**{k: v for k, v in inputs.items()}))
    data, consts, meta, perm = host_prep(**inputs)
    print("B max:", meta["B"].max(), "nblocks:", meta["nblocks"],
          "nchunks:", meta["nchunks"])
    outs = numpy_device_sim(data, consts, meta)
    got = assemble_output(outs, perm)
    err = np.abs(got - expected)
    rel = err.max() / np.abs(expected).max()
    print(f"numpy-sim max abs err {err.max():.3e}  rel {rel:.3e}")


# revision 10
# speedup vs baseline: 1.2111x; 1.0110x over previous
"""GCN (2-layer) Trainium2 kernel over 8 NeuronCores.

Strategy:
- GCN is node-permutation-equivariant: the host renumbers nodes so that
  (a) each core owns 6250 nodes (padded shard 6272 = 49 tiles x 128),
  (b) in-edge counts are balanced so that every (dst-tile, src-half) bucket
      holds <= 1024 edges -> exactly 8 gather-blocks of 128, no max-over-core
      padding.  This minimizes the GPSIMD SWDGE descriptor-generation time,
      which is the hard serial bottleneck (~8ns/edge on the Pool engine).
- h1 = (x @ W1) scaled by dinv (deg^-1/2) computed shard-local -> AllGather
  to a full 50176-row table in each core's DRAM.
- Scatter-add aggregation out[d] += table[src] over REAL edges only is done
  per dst-core: rows fetched with gpsimd.dma_gather (int16 idx over two
  25088-row halves); segmented sum per 128-dst tile via TensorE matmul with
  a DVE-built one-hot selector.  The self-loop term is added locally from
  the resident shard (no gather traffic).
- Layer 2 aggregates the dinv-scaled relu table (64 feats), then W2 + bias +
  log_softmax on-chip.
The edge structure is baked into the program at build time (SPMD; identical
program on all 8 cores, per-core data differs, padded to a common shape).
"""

import numpy as np

N_NODES = 50000
CORES = 8
SH = 6250          # owned nodes per core
SHP = 6272         # padded shard rows (49*128)
NT = 49            # dst tiles per core
HALF = SHP * 4     # 25088 table rows per half (cores 0-3 | 4-7)
F0, F1, F2 = 96, 64, 16
BLK = 128
CHUNK_BLOCKS = 8   # 1024 idx per dma_gather (single_packet limit)
CHUNK = BLK * CHUNK_BLOCKS
ZROW = 25087       # half-local row of a guaranteed-zero table row (both halves)


# --------------------------------------------------------------------------
# host-side balancing: nodes -> (core, tile, slot)
# --------------------------------------------------------------------------

def _balance(src, dst):
    """Assign nodes to cores/tiles/slots, balancing real-edge in-degrees so
    that each (core, tile, src-half) bucket holds close to 1024 edges."""
    N = N_NODES
    deg = np.bincount(dst, minlength=N).astype(np.int64)

    # ---- phase 1: core assignment, balance total in-degree, cap 6250 ----
    order = np.argsort(-deg, kind="stable")
    core_of = np.empty(N, np.int64)
    load = np.zeros(CORES, np.int64)
    cnt = np.zeros(CORES, np.int64)
    for v in order:
        c = -1
        best = None
        for k in range(CORES):
            if cnt[k] < SH and (best is None or load[k] < best):
                best = load[k]
                c = k
        core_of[v] = c
        load[c] += deg[v]
        cnt[c] += 1

    # ---- phase 2: balance A/B half split of each core's in-edges --------
    # O[u, c] = # out-edges of u landing in core c
    O = np.zeros((N, CORES), np.int32)
    np.add.at(O, (src, core_of[dst]), 1)
    in_tot = load.copy()  # in-edges per core

    def half_excess():
        # E(c, A) - in(c)/2 for each core c (A = src core < 4)
        srcA = core_of[src] < 4
        EA = np.bincount(core_of[dst][srcA], minlength=CORES)
        return EA - in_tot / 2.0

    delta = half_excess()
    isA = core_of < 4
    for _ in range(400):
        w = int(np.argmax(np.abs(delta)))
        if abs(delta[w]) <= 16:
            break
        # delta[w] > 0: too many in-edges of w from half A -> move a node
        # with many out-edges-into-w from A to B (swap with equal-degree).
        if delta[w] > 0:
            cand1 = np.where(isA)[0]
            cand2mask = ~isA
        else:
            cand1 = np.where(~isA)[0]
            cand2mask = isA
        u1 = cand1[np.argmax(O[cand1, w])]
        d1 = deg[u1]
        cand2 = np.where(cand2mask & (deg == d1))[0]
        if len(cand2) == 0:
            break
        u2 = cand2[np.argmin(O[cand2, w])]
        # swap cores of u1, u2 (equal in-degree keeps in_tot intact)
        c1, c2 = core_of[u1], core_of[u2]
        core_of[u1], core_of[u2] = c2, c1
        isA[u1], isA[u2] = c2 < 4, c1 < 4
        # update delta incrementally: u1 moved A->B (or B->A), u2 opposite
        s1 = 1.0 if c1 < 4 else -1.0  # u1 leaves half(c1)
        delta = delta - s1 * O[u1] + s1 * O[u2]

    # ---- phase 3: per-core tile packing, 2D (dA, dB) <= (1024, 1024) ----
    srcA = core_of[src] < 4
    dA = np.bincount(dst[srcA], minlength=N).astype(np.int64)
    dB = deg - dA

    tile_of = np.empty(N, np.int64)
    slot_of = np.empty(N, np.int64)
    for c in range(CORES):
        nodes = np.where(core_of == c)[0]
        nodes = nodes[np.argsort(-(dA[nodes] + dB[nodes]), kind="stable")]
        cap = np.full(NT, BLK, np.int64)
        if c in (3, 7):
            cap[NT - 1] = BLK - 1  # reserve the ZROW slot
        la = np.zeros(NT, np.int64)
        lb = np.zeros(NT, np.int64)
        nc_ = np.zeros(NT, np.int64)
        tl = np.empty(len(nodes), np.int64)
        for i, v in enumerate(nodes):
            score = np.maximum(la + dA[v], lb + dB[v]).astype(np.float64)
            score[nc_ >= cap] = np.inf
            t = int(np.argmin(score))
            tl[i] = t
            la[t] += dA[v]
            lb[t] += dB[v]
            nc_[t] += 1
        # repair: move nodes out of overfull bins (either half > 1024)
        for _ in range(600):
            over = np.where((la > CHUNK) | (lb > CHUNK))[0]
            if len(over) == 0:
                break
            t = int(over[0])
            halfsel = la if la[t] > CHUNK else lb
            dsel = dA if la[t] > CHUNK else dB
            members = np.where(tl == t)[0]
            excess = halfsel[t] - CHUNK
            diffs = dsel[nodes[members]]
            k = members[np.argmin(np.abs(diffs - excess))]
            v = nodes[k]
            score = np.maximum(la + dA[v], lb + dB[v]).astype(np.float64)
            score[nc_ >= cap] = np.inf
            score[t] = np.inf
            t2 = int(np.argmin(score))
            tl[k] = t2
            la[t] -= dA[v]; lb[t] -= dB[v]; nc_[t] -= 1
            la[t2] += dA[v]; lb[t2] += dB[v]; nc_[t2] += 1
        # swap repair: exchange nodes between overfull/underfull tiles
        for _ in range(400):
            over = np.where((la > CHUNK) | (lb > CHUNK))[0]
            if len(over) == 0:
                break
            t = int(over[0])
            useA = la[t] > CHUNK
            dsel = dA if useA else dB
            doth = dB if useA else dA
            lsel = la if useA else lb
            loth = lb if useA else la
            members = np.where(tl == t)[0]
            excess = lsel[t] - CHUNK
            done = False
            for k in members[np.argsort(-dsel[nodes[members]])]:
                v = nodes[k]
                # find a partner tile+node: swapping v with u reduces t's
                # overfull half without overflowing anything else
                for t2 in np.argsort(lsel)[:8]:
                    if t2 == t:
                        continue
                    mem2 = np.where(tl == t2)[0]
                    for k2 in mem2[np.argsort(dsel[nodes[mem2]])[:4]]:
                        u = nodes[k2]
                        gain = dsel[v] - dsel[u]
                        if gain < excess * 0 + 1:
                            continue
                        nlsel_t2 = lsel[t2] - dsel[u] + dsel[v]
                        nloth_t2 = loth[t2] - doth[u] + doth[v]
                        nloth_t = loth[t] - doth[v] + doth[u]
                        if (nlsel_t2 <= CHUNK and nloth_t2 <= CHUNK
                                and nloth_t <= CHUNK):
                            tl[k], tl[k2] = t2, t
                            la[t] += dA[u] - dA[v]; lb[t] += dB[u] - dB[v]
                            la[t2] += dA[v] - dA[u]; lb[t2] += dB[v] - dB[u]
                            done = True
                            break
                    if done:
                        break
                if done:
                    break
            if not done:
                break
        tile_of[nodes] = tl
        # assign slots within tiles in order
        for t in range(NT):
            members = nodes[tl == t]
            slot_of[members] = np.arange(len(members))

    return core_of, tile_of, slot_of


def host_prep(x, edge_index, W1, b1, W2, b2):
    src = np.asarray(edge_index[0], dtype=np.int64)
    dst = np.asarray(edge_index[1], dtype=np.int64)

    core_of, tile_of, slot_of = _balance(src, dst)
    row_of = core_of * SHP + tile_of * BLK + slot_of  # new table row per node

    deg_full = np.bincount(dst, minlength=N_NODES).astype(np.float32) + 1.0

    # per-edge fields
    ec = core_of[dst]
    et = tile_of[dst]
    edl = slot_of[dst]
    erow = row_of[src]
    eh = (erow >= HALF).astype(np.int64)
    elrow = erow - eh * HALF

    # B[t, h]: blocks per (tile, half) = global max over cores (normally 8)
    counts = np.zeros((CORES, NT, 2), np.int64)
    np.add.at(counts, (ec, et, eh), 1)
    B = np.maximum(1, -(-counts.max(axis=0) // BLK))  # [NT, 2]
    nblocks = [int(B[:, h].sum()) for h in (0, 1)]
    nchunks = [-(-nblocks[h] // CHUNK_BLOCKS) for h in (0, 1)]
    startgb = np.zeros((NT, 2), np.int64)
    acc = [0, 0]
    for t in range(NT):
        for h in (0, 1):
            startgb[t, h] = acc[h]
            acc[h] += B[t, h]

    # per-core streams
    okey = (ec * NT + et) * 2 + eh
    eorder = np.lexsort((elrow, okey))
    okey_s = okey[eorder]
    lrow_s = elrow[eorder]
    dl_s = edl[eorder]
    bounds = np.searchsorted(okey_s, np.arange(CORES * NT * 2 + 1))

    data = []
    for i in range(CORES):
        planes, dls = [], []
        for h in (0, 1):
            parts_i, parts_d = [], []
            for t in range(NT):
                k = (i * NT + t) * 2 + h
                a, b = bounds[k], bounds[k + 1]
                r, d = lrow_s[a:b], dl_s[a:b]
                pad = int(B[t, h]) * BLK - (b - a)
                assert pad >= 0, f"bucket overflow core{i} t{t} h{h}"
                parts_i.append(np.concatenate(
                    [r, np.full(pad, ZROW, np.int64)]))
                parts_d.append(np.concatenate([d, np.zeros(pad, np.int64)]))
            si = np.concatenate(parts_i)
            sd = np.concatenate(parts_d)
            tail = nchunks[h] * CHUNK - len(si)
            si = np.concatenate([si, np.full(tail, ZROW, np.int64)])
            pl = si.reshape(-1, 16).T.astype(np.int16)
            planes.append(np.tile(pl, (8, 1)))
            dls.append(np.ascontiguousarray(
                sd.reshape(-1, BLK).T.astype(np.float32)))

        # deg plane [128, NT] and occupancy mask
        nodes_i = np.where(core_of == i)[0]
        degp = np.ones((BLK, NT), np.float32)
        maskp = np.zeros((BLK, NT), np.float32)
        degp[slot_of[nodes_i], tile_of[nodes_i]] = deg_full[nodes_i]
        maskp[slot_of[nodes_i], tile_of[nodes_i]] = 1.0

        # xT shard [96, 6272]
        xs = np.zeros((F0, SHP), np.float32)
        xs[:, tile_of[nodes_i] * BLK + slot_of[nodes_i]] = \
            np.asarray(x, np.float32)[nodes_i].T
        import ml_dtypes
        data.append(dict(
            xT=np.ascontiguousarray(xs.astype(ml_dtypes.bfloat16)),
            idx0=np.ascontiguousarray(planes[0]),
            idx1=np.ascontiguousarray(planes[1]),
            dl0=np.ascontiguousarray(dls[0]),
            dl1=np.ascontiguousarray(dls[1]),
            deg=np.ascontiguousarray(degp),
            mask=np.ascontiguousarray(maskp),
        ))

    import ml_dtypes
    consts = dict(
        W1=np.asarray(W1, np.float32).astype(ml_dtypes.bfloat16),
        W2=np.asarray(W2, np.float32),
        b1b=np.tile(np.asarray(b1, np.float32), (BLK, 1)),
        b2b=np.tile(np.asarray(b2, np.float32), (BLK, 1)),
        iota=np.tile(np.arange(BLK, dtype=np.float32), (BLK, 1)),
        ident=np.eye(BLK, dtype=np.float32),
    )
    meta = dict(B=B, nblocks=nblocks, nchunks=nchunks, startgb=startgb)
    perm = dict(core_of=core_of, tile_of=tile_of, slot_of=slot_of)
    return data, consts, meta, perm


def numpy_device_sim(data, consts, meta):
    """Replay the device algorithm in numpy (host-side validation)."""
    B, startgb, nblocks = meta["B"], meta["startgb"], meta["nblocks"]
    tables, dinvs = [], []
    for i in range(CORES):
        d = data[i]
        dinv = 1.0 / np.sqrt(d["deg"])
        dinvs.append(dinv)
        h = d["xT"].T @ consts["W1"]
        hs = h.reshape(NT, BLK, F1) * dinv.T[:, :, None]
        tables.append(hs.reshape(SHP, F1))
    table = np.concatenate(tables, 0)

    def layer(table, d):
        halves = [table[:HALF], table[HALF:]]
        agg = np.zeros((NT, F1, BLK), np.float32)
        for h in (0, 1):
            plane = d["idx0"] if h == 0 else d["idx1"]
            dl = d["dl0"] if h == 0 else d["dl1"]
            stream = plane[:16].T.reshape(-1)
            for t in range(NT):
                for b_ in range(int(B[t, h])):
                    gb = int(startgb[t, h]) + b_
                    rows = stream[gb * BLK:(gb + 1) * BLK].astype(np.int64)
                    G = halves[h][rows]
                    dloc = dl[:, gb].astype(np.int64)
                    S = np.zeros((BLK, BLK), np.float32)
                    S[np.arange(BLK), dloc] = 1.0
                    agg[t] += G.T @ S
        return agg

    full2 = []
    for i in range(CORES):
        d = data[i]
        agg = layer(table, d)
        dinv, mask = dinvs[i], d["mask"]
        tloc = tables[i].reshape(NT, BLK, F1)
        t2 = []
        for t in range(NT):
            a = agg[t].T + tloc[t]  # self term
            e = np.maximum(a * dinv[:, t:t + 1] + consts["b1b"], 0.0) \
                * (dinv[:, t:t + 1] * mask[:, t:t + 1])
            t2.append(e)
        full2.append(np.stack(t2).reshape(SHP, F1))
    table2 = np.concatenate(full2, 0)

    outs = []
    for i in range(CORES):
        d = data[i]
        agg = layer(table2, d)
        dinv = dinvs[i]
        t2loc = full2[i].reshape(NT, BLK, F1)
        o = np.zeros((NT, BLK, F2), np.float32)
        for t in range(NT):
            a = (agg[t].T + t2loc[t]) * dinv[:, t:t + 1]
            z = a @ consts["W2"] + consts["b2b"]
            m = z.max(1, keepdims=True)
            o[t] = z - m - np.log(np.exp(z - m).sum(1, keepdims=True))
        outs.append(o.reshape(SHP, F2))
    return np.stack(outs)


def assemble_output(outs, perm):
    res = np.zeros((N_NODES, F2), np.float32)
    rows = perm["tile_of"] * BLK + perm["slot_of"]
    for i in range(CORES):
        sel = perm["core_of"] == i
        res[np.where(sel)[0]] = outs[i][rows[sel]]
    return res


def build_nc(meta):
    import concourse.bacc as bacc
    import concourse.tile as tile
    import concourse.mybir as mybir

    dt = mybir.dt.float32
    Alu = mybir.AluOpType
    Act = mybir.ActivationFunctionType
    B, nblocks, nchunks, startgb = (
        meta["B"], meta["nblocks"], meta["nchunks"], meta["startgb"])

    nc = bacc.Bacc(None, target_bir_lowering=False)
    p_xT = nc.declare_dram_parameter("xT", [F0, SHP], mybir.dt.bfloat16,
                                     isOutput=False)
    p_idx = [nc.declare_dram_parameter(f"idx{h}", [128, nchunks[h] * (CHUNK // 16)],
                                       mybir.dt.int16, isOutput=False) for h in (0, 1)]
    p_dl = [nc.declare_dram_parameter(f"dl{h}", [128, nblocks[h]], dt, isOutput=False)
            for h in (0, 1)]
    p_deg = nc.declare_dram_parameter("deg", [128, NT], dt, isOutput=False)
    p_mask = nc.declare_dram_parameter("mask", [128, NT], dt, isOutput=False)
    p_W1 = nc.declare_dram_parameter("W1", [F0, F1], mybir.dt.bfloat16,
                                     isOutput=False)
    p_W2 = nc.declare_dram_parameter("W2", [F1, F2], dt, isOutput=False)
    p_b1 = nc.declare_dram_parameter("b1b", [128, F1], dt, isOutput=False)
    p_b2 = nc.declare_dram_parameter("b2b", [128, F2], dt, isOutput=False)
    p_iota = nc.declare_dram_parameter("iota", [128, 128], dt, isOutput=False)
    p_ident = nc.declare_dram_parameter("ident", [128, 128], dt, isOutput=False)
    p_out = nc.declare_dram_parameter("out", [128, NT * F2], dt, isOutput=True)

    cc_in = [nc.dram_tensor(f"cc_in{li}", [SHP, F1], dt) for li in (0, 1)]
    cc_out = [nc.dram_tensor(f"cc_out{li}", [CORES * SHP, F1], dt, addr_space="Shared")
              for li in (0, 1)]

    with tile.TileContext(nc) as tc:
        with (
            tc.tile_pool(name="cpool", bufs=1) as cpool,
            tc.tile_pool(name="spool", bufs=4) as spool,
            tc.tile_pool(name="stpool", bufs=8) as stpool,
            tc.tile_pool(name="wpool", bufs=4) as wpool,
            tc.tile_pool(name="ppool", bufs=3, space="PSUM") as ppool,
            tc.tile_pool(name="popool", bufs=2, space="PSUM") as popool,
        ):
            # ---- constants into SBUF
            xT = cpool.tile([F0, SHP], mybir.dt.bfloat16)
            nc.sync.dma_start(xT[:], p_xT[:])
            W1 = cpool.tile([F0, F1], mybir.dt.bfloat16)
            nc.sync.dma_start(W1[:], p_W1[:])
            degt = cpool.tile([128, NT], dt)
            nc.sync.dma_start(degt[:], p_deg[:])
            maskt = cpool.tile([128, NT], dt)
            nc.sync.dma_start(maskt[:], p_mask[:])

            recd = cpool.tile([128, NT], dt)
            nc.vector.reciprocal(recd[:], degt[:])
            dinv = cpool.tile([128, NT], dt)
            nc.scalar.activation(dinv[:], recd[:], Act.Sqrt)
            dinvm = cpool.tile([128, NT], dt)
            nc.vector.tensor_tensor(out=dinvm[:], in0=dinv[:], in1=maskt[:],
                                    op=Alu.mult)

            # ---- head: T1 shard = dinv * (x @ W1)
            Tsh = cpool.tile([128, NT * F1], dt)
            for t in range(NT):
                psh = ppool.tile([128, F1], dt, tag="agg1", name=f"psh{t}")
                nc.tensor.matmul(psh[:], xT[:, BLK * t:BLK * (t + 1)], W1[:],
                                 start=True, stop=True)
                nc.vector.tensor_scalar(
                    Tsh[:, F1 * t:F1 * (t + 1)], psh[:], dinv[:, t:t + 1], None,
                    Alu.mult)
                nc.sync.dma_start(cc_in[0][BLK * t:BLK * (t + 1), :],
                                  Tsh[:, F1 * t:F1 * (t + 1)])
            nc.gpsimd.collective_compute(
                "AllGather", Alu.bypass,
                ins=[cc_in[0].ap().opt()], outs=[cc_out[0].ap().opt()],
                replica_groups=[list(range(CORES))])

            # deferred constant loads (needed only after AllGather #1)
            W2 = cpool.tile([F1, F2], dt)
            nc.sync.dma_start(W2[:], p_W2[:])
            b1b = cpool.tile([128, F1], dt)
            nc.sync.dma_start(b1b[:], p_b1[:])
            b2b = cpool.tile([128, F2], dt)
            nc.sync.dma_start(b2b[:], p_b2[:])
            iota = cpool.tile([128, 128], dt)
            nc.sync.dma_start(iota[:], p_iota[:])
            ident = cpool.tile([128, 128], dt)
            nc.sync.dma_start(ident[:], p_ident[:])
            idx_sb = []
            dl_sb = []
            for h in (0, 1):
                isb = cpool.tile([128, nchunks[h] * (CHUNK // 16)], mybir.dt.int16,
                                 name=f"isb{h}")
                nc.sync.dma_start(isb[:], p_idx[h][:])
                idx_sb.append(isb)
                dsb = cpool.tile([128, nblocks[h]], dt, name=f"dsb{h}")
                nc.sync.dma_start(dsb[:], p_dl[h][:])
                dl_sb.append(dsb)

            def do_layer(li, table, self_tab, tail_fn):
                halves = [table[0:HALF, :], table[HALF:2 * HALF, :]]
                emitted = [0, 0]
                chunks = [{}, {}]

                def ensure_chunk(h, c):
                    while emitted[h] <= min(c + 3, nchunks[h] - 1):
                        ce = emitted[h]
                        st = stpool.tile([128, CHUNK_BLOCKS, F1], dt,
                                         tag=f"st{h}", name=f"st_l{li}_h{h}_c{ce}")
                        cols = CHUNK // 16
                        nc.gpsimd.dma_gather(
                            st[:], halves[h], idx_sb[h][:, ce * cols:(ce + 1) * cols],
                            CHUNK, CHUNK, F1)
                        chunks[h][ce] = st
                        emitted[h] += 1
                    return chunks[h][c]

                for t in range(NT):
                    nb = int(B[t, 0] + B[t, 1])
                    pagg = ppool.tile([128, F1], dt, tag="agg1", name=f"pg{li}_{t}")
                    k = 0
                    for h in (0, 1):
                        for b in range(int(B[t, h])):
                            gb = int(startgb[t, h]) + b
                            c, slot = gb // CHUNK_BLOCKS, gb % CHUNK_BLOCKS
                            st = ensure_chunk(h, c)
                            S = spool.tile([128, 128], dt, tag="S",
                                           name=f"S{li}_{t}_{h}_{b}")
                            nc.vector.tensor_scalar(
                                S[:], iota[:], dl_sb[h][:, gb:gb + 1], None,
                                Alu.is_equal)
                            nc.tensor.matmul(pagg[:], S[:], st[:, slot, :],
                                             start=(k == 0), stop=(k == nb - 1))
                            k += 1
                    tail_fn(t, pagg)

            # ---- layer 1
            T2sh = cpool.tile([128, NT * F1], dt)

            def tail1(t, pagg):
                e0 = wpool.tile([128, F1], dt, tag="e0", name=f"e0_{t}")
                nc.vector.tensor_tensor(out=e0[:], in0=pagg[:],
                                        in1=Tsh[:, F1 * t:F1 * (t + 1)], op=Alu.add)
                e1 = wpool.tile([128, F1], dt, tag="e1", name=f"e1_{t}")
                nc.vector.tensor_scalar(e1[:], e0[:], dinv[:, t:t + 1], None,
                                        Alu.mult)
                e2 = wpool.tile([128, F1], dt, tag="e2", name=f"e2_{t}")
                nc.vector.tensor_tensor(out=e2[:], in0=e1[:], in1=b1b[:], op=Alu.add)
                nc.vector.tensor_scalar(
                    T2sh[:, F1 * t:F1 * (t + 1)], e2[:], 0.0, dinvm[:, t:t + 1],
                    Alu.max, Alu.mult)
                nc.sync.dma_start(cc_in[1][BLK * t:BLK * (t + 1), :],
                                  T2sh[:, F1 * t:F1 * (t + 1)])

            do_layer(0, cc_out[0], None, tail1)
            nc.gpsimd.collective_compute(
                "AllGather", Alu.bypass,
                ins=[cc_in[1].ap().opt()], outs=[cc_out[1].ap().opt()],
                replica_groups=[list(range(CORES))])

            # ---- layer 2

            def tail2(t, pagg):
                a0 = wpool.tile([128, F1], dt, tag="a0", name=f"a0_{t}")
                nc.vector.tensor_tensor(out=a0[:], in0=pagg[:],
                                        in1=T2sh[:, F1 * t:F1 * (t + 1)],
                                        op=Alu.add)
                pt = popool.tile([F1, BLK], dt, tag="pt", name=f"pt_{t}")
                nc.tensor.matmul(pt[:], a0[:], ident[:], start=True, stop=True)
                aggS = wpool.tile([F1, BLK], dt, tag="aggS", name=f"as_{t}")
                nc.vector.tensor_copy(aggS[:], pt[:])
                po = popool.tile([128, F2], dt, tag="po", name=f"po_{t}")
                nc.tensor.matmul(po[:], aggS[:], W2[:], start=True, stop=True)
                e3 = wpool.tile([128, F2], dt, tag="e3", name=f"e3_{t}")
                nc.vector.tensor_scalar(e3[:], po[:], dinv[:, t:t + 1], None,
                                        Alu.mult)
                e4 = wpool.tile([128, F2], dt, tag="e4", name=f"e4_{t}")
                nc.vector.tensor_tensor(out=e4[:], in0=e3[:], in1=b2b[:], op=Alu.add)
                m = wpool.tile([128, 1], dt, tag="m", name=f"m_{t}")
                nc.vector.tensor_reduce(m[:], e4[:], axis=mybir.AxisListType.X,
                                        op=Alu.max)
                nm = wpool.tile([128, 1], dt, tag="nm", name=f"nm_{t}")
                nc.vector.tensor_scalar(nm[:], m[:], -1.0, None, Alu.mult)
                ex = wpool.tile([128, F2], dt, tag="ex", name=f"ex_{t}")
                nc.scalar.activation(ex[:], e4[:], Act.Exp, bias=nm[:, 0:1])
                sm = wpool.tile([128, 1], dt, tag="sm", name=f"sm_{t}")
                nc.vector.tensor_reduce(sm[:], ex[:], axis=mybir.AxisListType.X,
                                        op=Alu.add)
                lg = wpool.tile([128, 1], dt, tag="lg", name=f"lg_{t}")
                nc.scalar.activation(lg[:], sm[:], Act.Ln)
                fo = wpool.tile([128, F2], dt, tag="fo", name=f"fo_{t}")
                nc.vector.tensor_scalar(
                    fo[:], e4[:], m[:, 0:1], lg[:, 0:1],
                    Alu.subtract, Alu.subtract)
                nc.sync.dma_start(p_out[:, F2 * t:F2 * (t + 1)], fo[:])

            def tail2_scaled(t, pagg):
                # pagg already includes the transposed self term; note the
                # dinv scale applies AFTER adding self (both are pre-scaled
                # table rows), matching tail2's e3 step.
                tail2(t, pagg)

            do_layer(1, cc_out[1], T2sh, tail2_scaled)

    nc.finalize()
    return nc


LAST_EXEC_NS = None


def kernel(x, edge_index, W1, b1, W2, b2):
    from concourse.bass_utils import run_bass_kernel_spmd

    x = np.asarray(x, np.float32)
    data, consts, meta, perm = host_prep(x, np.asarray(edge_index), W1, b1, W2, b2)
    nc = build_nc(meta)
    in_maps = []
    for i in range(CORES):
        m = dict(data[i])
        m.update({k: np.ascontiguousarray(v) for k, v in consts.items()})
        in_maps.append(m)
    import os as _os
    trace = bool(int(_os.environ.get("GCN_TRACE", "0")))
    res = run_bass_kernel_spmd(nc, in_maps, core_ids=list(range(CORES)), trace=trace)
    global LAST_EXEC_NS
    LAST_EXEC_NS = res.exec_time_ns
    outs = []
    for i in range(CORES):
        o = res.results[i]["out"]  # [128, NT*F2]
        outs.append(o.reshape(128, NT, F2).transpose(1, 0, 2).reshape(SHP, F2))
    return assemble_output(np.stack(outs), perm)


if __name__ == "__main__":
    import reference
    inputs = {k: np.asarray(v) for k, v in reference.setup_inputs().items()}
    expected = np.asarray(reference.reference(**{k: v for k, v in inputs.items()}))
    data, consts, meta, perm = host_prep(**inputs)
    print("B max:", meta["B"].max(), "nblocks:", meta["nblocks"],
          "nchunks:", meta["nchunks"])
    outs = numpy_device_sim(data, consts, meta)
    got = assemble_output(outs, perm)
    err = np.abs(got - expected)
    rel = err.max() / np.abs(expected).max()
    print(f"numpy-sim max abs err {err.max():.3e}  rel {rel:.3e}")


# revision 12
# speedup vs baseline: 1.2991x; 1.0727x over previous
"""GCN (2-layer) Trainium2 kernel over 8 NeuronCores.

Strategy:
- GCN is node-permutation-equivariant: the host renumbers nodes so that
  (a) each core owns 6250 nodes (padded shard 6272 = 49 tiles x 128),
  (b) in-edge counts are balanced so that every (dst-tile, src-half) bucket
      holds <= 1024 edges -> exactly 8 gather-blocks of 128, no max-over-core
      padding.  This minimizes the GPSIMD SWDGE descriptor-generation time,
      which is the hard serial bottleneck (~8ns/edge on the Pool engine).
- h1 = (x @ W1) scaled by dinv (deg^-1/2) computed shard-local -> AllGather
  to a full 50176-row table in each core's DRAM.
- Scatter-add aggregation out[d] += table[src] over REAL edges only is done
  per dst-core: rows fetched with gpsimd.dma_gather (int16 idx over two
  25088-row halves); segmented sum per 128-dst tile via TensorE matmul with
  a DVE-built one-hot selector.  The self-loop term is added locally from
  the resident shard (no gather traffic).
- Layer 2 aggregates the dinv-scaled relu table (64 feats), then W2 + bias +
  log_softmax on-chip.
The edge structure is baked into the program at build time (SPMD; identical
program on all 8 cores, per-core data differs, padded to a common shape).
"""

import numpy as np

N_NODES = 50000
CORES = 8
SH = 6250          # owned nodes per core
SHP = 6272         # padded shard rows (49*128)
NT = 49            # dst tiles per core
HALF = SHP * 4     # 25088 table rows per half (cores 0-3 | 4-7)
F0, F1, F2 = 96, 64, 16
BLK = 128
CHUNK_BLOCKS = 8   # 1024 idx per dma_gather (single_packet limit)
CHUNK = BLK * CHUNK_BLOCKS
ZROW = 25087       # half-local row of a guaranteed-zero table row (both halves)
SPLIT_T = 25       # head/tail tile count before the first sub-AllGather
SPLIT_R = SPLIT_T * BLK


# --------------------------------------------------------------------------
# host-side balancing: nodes -> (core, tile, slot)
# --------------------------------------------------------------------------

def _balance(src, dst):
    """Assign nodes to cores/tiles/slots, balancing real-edge in-degrees so
    that each (core, tile, src-half) bucket holds close to 1024 edges."""
    N = N_NODES
    deg = np.bincount(dst, minlength=N).astype(np.int64)

    # ---- phase 1: core assignment, balance total in-degree, cap 6250 ----
    order = np.argsort(-deg, kind="stable")
    core_of = np.empty(N, np.int64)
    load = np.zeros(CORES, np.int64)
    cnt = np.zeros(CORES, np.int64)
    for v in order:
        c = -1
        best = None
        for k in range(CORES):
            if cnt[k] < SH and (best is None or load[k] < best):
                best = load[k]
                c = k
        core_of[v] = c
        load[c] += deg[v]
        cnt[c] += 1

    # ---- phase 2: balance A/B half split of each core's in-edges --------
    # O[u, c] = # out-edges of u landing in core c
    O = np.zeros((N, CORES), np.int32)
    np.add.at(O, (src, core_of[dst]), 1)
    in_tot = load.copy()  # in-edges per core

    def half_excess():
        # E(c, A) - in(c)/2 for each core c (A = src core < 4)
        srcA = core_of[src] < 4
        EA = np.bincount(core_of[dst][srcA], minlength=CORES)
        return EA - in_tot / 2.0

    delta = half_excess()
    isA = core_of < 4
    for _ in range(400):
        w = int(np.argmax(np.abs(delta)))
        if abs(delta[w]) <= 16:
            break
        # delta[w] > 0: too many in-edges of w from half A -> move a node
        # with many out-edges-into-w from A to B (swap with equal-degree).
        if delta[w] > 0:
            cand1 = np.where(isA)[0]
            cand2mask = ~isA
        else:
            cand1 = np.where(~isA)[0]
            cand2mask = isA
        u1 = cand1[np.argmax(O[cand1, w])]
        d1 = deg[u1]
        cand2 = np.where(cand2mask & (deg == d1))[0]
        if len(cand2) == 0:
            break
        u2 = cand2[np.argmin(O[cand2, w])]
        # swap cores of u1, u2 (equal in-degree keeps in_tot intact)
        c1, c2 = core_of[u1], core_of[u2]
        core_of[u1], core_of[u2] = c2, c1
        isA[u1], isA[u2] = c2 < 4, c1 < 4
        # update delta incrementally: u1 moved A->B (or B->A), u2 opposite
        s1 = 1.0 if c1 < 4 else -1.0  # u1 leaves half(c1)
        delta = delta - s1 * O[u1] + s1 * O[u2]

    # ---- phase 3: per-core tile packing, 2D (dA, dB) <= (1024, 1024) ----
    srcA = core_of[src] < 4
    dA = np.bincount(dst[srcA], minlength=N).astype(np.int64)
    dB = deg - dA

    tile_of = np.empty(N, np.int64)
    slot_of = np.empty(N, np.int64)
    for c in range(CORES):
        nodes = np.where(core_of == c)[0]
        nodes = nodes[np.argsort(-(dA[nodes] + dB[nodes]), kind="stable")]
        cap = np.full(NT, BLK, np.int64)
        if c in (3, 7):
            cap[NT - 1] = BLK - 1  # reserve the ZROW slot
        la = np.zeros(NT, np.int64)
        lb = np.zeros(NT, np.int64)
        nc_ = np.zeros(NT, np.int64)
        tl = np.empty(len(nodes), np.int64)
        for i, v in enumerate(nodes):
            score = np.maximum(la + dA[v], lb + dB[v]).astype(np.float64)
            score[nc_ >= cap] = np.inf
            t = int(np.argmin(score))
            tl[i] = t
            la[t] += dA[v]
            lb[t] += dB[v]
            nc_[t] += 1
        # repair: move nodes out of overfull bins (either half > 1024)
        for _ in range(600):
            over = np.where((la > CHUNK) | (lb > CHUNK))[0]
            if len(over) == 0:
                break
            t = int(over[0])
            halfsel = la if la[t] > CHUNK else lb
            dsel = dA if la[t] > CHUNK else dB
            members = np.where(tl == t)[0]
            excess = halfsel[t] - CHUNK
            diffs = dsel[nodes[members]]
            k = members[np.argmin(np.abs(diffs - excess))]
            v = nodes[k]
            score = np.maximum(la + dA[v], lb + dB[v]).astype(np.float64)
            score[nc_ >= cap] = np.inf
            score[t] = np.inf
            t2 = int(np.argmin(score))
            tl[k] = t2
            la[t] -= dA[v]; lb[t] -= dB[v]; nc_[t] -= 1
            la[t2] += dA[v]; lb[t2] += dB[v]; nc_[t2] += 1
        # swap repair: exchange nodes between overfull/underfull tiles
        for _ in range(400):
            over = np.where((la > CHUNK) | (lb > CHUNK))[0]
            if len(over) == 0:
                break
            t = int(over[0])
            useA = la[t] > CHUNK
            dsel = dA if useA else dB
            doth = dB if useA else dA
            lsel = la if useA else lb
            loth = lb if useA else la
            members = np.where(tl == t)[0]
            excess = lsel[t] - CHUNK
            done = False
            for k in members[np.argsort(-dsel[nodes[members]])]:
                v = nodes[k]
                # find a partner tile+node: swapping v with u reduces t's
                # overfull half without overflowing anything else
                for t2 in np.argsort(lsel)[:8]:
                    if t2 == t:
                        continue
                    mem2 = np.where(tl == t2)[0]
                    for k2 in mem2[np.argsort(dsel[nodes[mem2]])[:4]]:
                        u = nodes[k2]
                        gain = dsel[v] - dsel[u]
                        if gain < excess * 0 + 1:
                            continue
                        nlsel_t2 = lsel[t2] - dsel[u] + dsel[v]
                        nloth_t2 = loth[t2] - doth[u] + doth[v]
                        nloth_t = loth[t] - doth[v] + doth[u]
                        if (nlsel_t2 <= CHUNK and nloth_t2 <= CHUNK
                                and nloth_t <= CHUNK):
                            tl[k], tl[k2] = t2, t
                            la[t] += dA[u] - dA[v]; lb[t] += dB[u] - dB[v]
                            la[t2] += dA[v] - dA[u]; lb[t2] += dB[v] - dB[u]
                            done = True
                            break
                    if done:
                        break
                if done:
                    break
            if not done:
                break
        tile_of[nodes] = tl
        # assign slots within tiles in order
        for t in range(NT):
            members = nodes[tl == t]
            slot_of[members] = np.arange(len(members))

    return core_of, tile_of, slot_of


def host_prep(x, edge_index, W1, b1, W2, b2):
    src = np.asarray(edge_index[0], dtype=np.int64)
    dst = np.asarray(edge_index[1], dtype=np.int64)

    core_of, tile_of, slot_of = _balance(src, dst)
    row_of = core_of * SHP + tile_of * BLK + slot_of  # new table row per node

    deg_full = np.bincount(dst, minlength=N_NODES).astype(np.float32) + 1.0

    # per-edge fields
    ec = core_of[dst]
    et = tile_of[dst]
    edl = slot_of[dst]
    erow = row_of[src]
    eh = (erow >= HALF).astype(np.int64)
    elrow = erow - eh * HALF

    # B[t, h]: blocks per (tile, half) = global max over cores (normally 8)
    counts = np.zeros((CORES, NT, 2), np.int64)
    np.add.at(counts, (ec, et, eh), 1)
    B = np.maximum(1, -(-counts.max(axis=0) // BLK))  # [NT, 2]
    nblocks = [int(B[:, h].sum()) for h in (0, 1)]
    nchunks = [-(-nblocks[h] // CHUNK_BLOCKS) for h in (0, 1)]
    startgb = np.zeros((NT, 2), np.int64)
    acc = [0, 0]
    for t in range(NT):
        for h in (0, 1):
            startgb[t, h] = acc[h]
            acc[h] += B[t, h]

    # per-core streams
    okey = (ec * NT + et) * 2 + eh
    eorder = np.lexsort((elrow, okey))
    okey_s = okey[eorder]
    lrow_s = elrow[eorder]
    dl_s = edl[eorder]
    bounds = np.searchsorted(okey_s, np.arange(CORES * NT * 2 + 1))

    data = []
    for i in range(CORES):
        planes, dls = [], []
        for h in (0, 1):
            parts_i, parts_d = [], []
            for t in range(NT):
                k = (i * NT + t) * 2 + h
                a, b = bounds[k], bounds[k + 1]
                r, d = lrow_s[a:b], dl_s[a:b]
                pad = int(B[t, h]) * BLK - (b - a)
                assert pad >= 0, f"bucket overflow core{i} t{t} h{h}"
                parts_i.append(np.concatenate(
                    [r, np.full(pad, ZROW, np.int64)]))
                parts_d.append(np.concatenate([d, np.zeros(pad, np.int64)]))
            si = np.concatenate(parts_i)
            sd = np.concatenate(parts_d)
            tail = nchunks[h] * CHUNK - len(si)
            si = np.concatenate([si, np.full(tail, ZROW, np.int64)])
            pl = si.reshape(-1, 16).T.astype(np.int16)
            planes.append(np.tile(pl, (8, 1)))
            dls.append(np.ascontiguousarray(
                sd.reshape(-1, BLK).T.astype(np.float32)))

        # deg plane [128, NT] and occupancy mask
        nodes_i = np.where(core_of == i)[0]
        degp = np.ones((BLK, NT), np.float32)
        maskp = np.zeros((BLK, NT), np.float32)
        degp[slot_of[nodes_i], tile_of[nodes_i]] = deg_full[nodes_i]
        maskp[slot_of[nodes_i], tile_of[nodes_i]] = 1.0

        # xT shard [96, 6272]
        xs = np.zeros((F0, SHP), np.float32)
        xs[:, tile_of[nodes_i] * BLK + slot_of[nodes_i]] = \
            np.asarray(x, np.float32)[nodes_i].T
        import ml_dtypes
        data.append(dict(
            xT=np.ascontiguousarray(xs.astype(ml_dtypes.bfloat16)),
            idx0=np.ascontiguousarray(planes[0]),
            idx1=np.ascontiguousarray(planes[1]),
            dl0=np.ascontiguousarray(dls[0]),
            dl1=np.ascontiguousarray(dls[1]),
            deg=np.ascontiguousarray(degp),
            mask=np.ascontiguousarray(maskp),
        ))

    import ml_dtypes
    consts = dict(
        W1=np.asarray(W1, np.float32).astype(ml_dtypes.bfloat16),
        W2=np.asarray(W2, np.float32),
        b1b=np.tile(np.asarray(b1, np.float32), (BLK, 1)),
        b2b=np.tile(np.asarray(b2, np.float32), (BLK, 1)),
        iota=np.tile(np.arange(BLK, dtype=np.float32), (BLK, 1)),
        ident=np.eye(BLK, dtype=np.float32),
    )
    meta = dict(B=B, nblocks=nblocks, nchunks=nchunks, startgb=startgb)
    perm = dict(core_of=core_of, tile_of=tile_of, slot_of=slot_of)
    return data, consts, meta, perm


def numpy_device_sim(data, consts, meta):
    """Replay the device algorithm in numpy (host-side validation)."""
    B, startgb, nblocks = meta["B"], meta["startgb"], meta["nblocks"]
    tables, dinvs = [], []
    for i in range(CORES):
        d = data[i]
        dinv = 1.0 / np.sqrt(d["deg"])
        dinvs.append(dinv)
        h = d["xT"].T @ consts["W1"]
        hs = h.reshape(NT, BLK, F1) * dinv.T[:, :, None]
        tables.append(hs.reshape(SHP, F1))
    table = np.concatenate(tables, 0)

    def layer(table, d):
        halves = [table[:HALF], table[HALF:]]
        agg = np.zeros((NT, F1, BLK), np.float32)
        for h in (0, 1):
            plane = d["idx0"] if h == 0 else d["idx1"]
            dl = d["dl0"] if h == 0 else d["dl1"]
            stream = plane[:16].T.reshape(-1)
            for t in range(NT):
                for b_ in range(int(B[t, h])):
                    gb = int(startgb[t, h]) + b_
                    rows = stream[gb * BLK:(gb + 1) * BLK].astype(np.int64)
                    G = halves[h][rows]
                    dloc = dl[:, gb].astype(np.int64)
                    S = np.zeros((BLK, BLK), np.float32)
                    S[np.arange(BLK), dloc] = 1.0
                    agg[t] += G.T @ S
        return agg

    full2 = []
    for i in range(CORES):
        d = data[i]
        agg = layer(table, d)
        dinv, mask = dinvs[i], d["mask"]
        tloc = tables[i].reshape(NT, BLK, F1)
        t2 = []
        for t in range(NT):
            a = agg[t].T + tloc[t]  # self term
            e = np.maximum(a * dinv[:, t:t + 1] + consts["b1b"], 0.0) \
                * (dinv[:, t:t + 1] * mask[:, t:t + 1])
            t2.append(e)
        full2.append(np.stack(t2).reshape(SHP, F1))
    table2 = np.concatenate(full2, 0)

    outs = []
    for i in range(CORES):
        d = data[i]
        agg = layer(table2, d)
        dinv = dinvs[i]
        t2loc = full2[i].reshape(NT, BLK, F1)
        o = np.zeros((NT, BLK, F2), np.float32)
        for t in range(NT):
            a = (agg[t].T + t2loc[t]) * dinv[:, t:t + 1]
            z = a @ consts["W2"] + consts["b2b"]
            m = z.max(1, keepdims=True)
            o[t] = z - m - np.log(np.exp(z - m).sum(1, keepdims=True))
        outs.append(o.reshape(SHP, F2))
    return np.stack(outs)


def assemble_output(outs, perm):
    res = np.zeros((N_NODES, F2), np.float32)
    rows = perm["tile_of"] * BLK + perm["slot_of"]
    for i in range(CORES):
        sel = perm["core_of"] == i
        res[np.where(sel)[0]] = outs[i][rows[sel]]
    return res


def build_nc(meta):
    import concourse.bacc as bacc
    import concourse.tile as tile
    import concourse.mybir as mybir

    dt = mybir.dt.float32
    Alu = mybir.AluOpType
    Act = mybir.ActivationFunctionType
    B, nblocks, nchunks, startgb = (
        meta["B"], meta["nblocks"], meta["nchunks"], meta["startgb"])

    nc = bacc.Bacc(None, target_bir_lowering=False)
    p_xT = nc.declare_dram_parameter("xT", [F0, SHP], mybir.dt.bfloat16,
                                     isOutput=False)
    p_idx = [nc.declare_dram_parameter(f"idx{h}", [128, nchunks[h] * (CHUNK // 16)],
                                       mybir.dt.int16, isOutput=False) for h in (0, 1)]
    p_dl = [nc.declare_dram_parameter(f"dl{h}", [128, nblocks[h]], dt, isOutput=False)
            for h in (0, 1)]
    p_deg = nc.declare_dram_parameter("deg", [128, NT], dt, isOutput=False)
    p_mask = nc.declare_dram_parameter("mask", [128, NT], dt, isOutput=False)
    p_W1 = nc.declare_dram_parameter("W1", [F0, F1], mybir.dt.bfloat16,
                                     isOutput=False)
    p_W2 = nc.declare_dram_parameter("W2", [F1, F2], dt, isOutput=False)
    p_b1 = nc.declare_dram_parameter("b1b", [128, F1], dt, isOutput=False)
    p_b2 = nc.declare_dram_parameter("b2b", [128, F2], dt, isOutput=False)
    p_iota = nc.declare_dram_parameter("iota", [128, 128], dt, isOutput=False)
    p_ident = nc.declare_dram_parameter("ident", [128, 128], dt, isOutput=False)
    p_out = nc.declare_dram_parameter("out", [128, NT * F2], dt, isOutput=True)

    cc_in = [nc.dram_tensor(f"cc_in{li}", [SHP, F1], dt) for li in (0, 1)]
    cc_out = [nc.dram_tensor(f"cc_out{li}", [CORES * SHP, F1], dt, addr_space="Shared")
              for li in (0, 1)]

    with tile.TileContext(nc) as tc:
        with (
            tc.tile_pool(name="cpool", bufs=1) as cpool,
            tc.tile_pool(name="spool", bufs=6) as spool,
            tc.tile_pool(name="stpool", bufs=9) as stpool,
            tc.tile_pool(name="wpool", bufs=4) as wpool,
            tc.tile_pool(name="ppool", bufs=3, space="PSUM") as ppool,
            tc.tile_pool(name="popool", bufs=2, space="PSUM") as popool,
        ):
            # ---- constants into SBUF
            xT = cpool.tile([F0, SHP], mybir.dt.bfloat16)
            nc.sync.dma_start(xT[:], p_xT[:])
            W1 = cpool.tile([F0, F1], mybir.dt.bfloat16)
            nc.sync.dma_start(W1[:], p_W1[:])
            degt = cpool.tile([128, NT], dt)
            nc.sync.dma_start(degt[:], p_deg[:])
            maskt = cpool.tile([128, NT], dt)
            nc.sync.dma_start(maskt[:], p_mask[:])

            recd = cpool.tile([128, NT], dt)
            nc.vector.reciprocal(recd[:], degt[:])
            dinv = cpool.tile([128, NT], dt)
            nc.scalar.activation(dinv[:], recd[:], Act.Sqrt)
            dinvm = cpool.tile([128, NT], dt)
            nc.vector.tensor_tensor(out=dinvm[:], in0=dinv[:], in1=maskt[:],
                                    op=Alu.mult)

            # ---- head: T1 shard = dinv * (x @ W1)
            Tsh = cpool.tile([128, NT * F1], dt)
            for t in range(NT):
                psh = ppool.tile([128, F1], dt, tag="agg1", name=f"psh{t}")
                nc.tensor.matmul(psh[:], xT[:, BLK * t:BLK * (t + 1)], W1[:],
                                 start=True, stop=True)
                nc.vector.tensor_scalar(
                    Tsh[:, F1 * t:F1 * (t + 1)], psh[:], dinv[:, t:t + 1], None,
                    Alu.mult)
                nc.sync.dma_start(cc_in[0][BLK * t:BLK * (t + 1), :],
                                  Tsh[:, F1 * t:F1 * (t + 1)])
            nc.gpsimd.collective_compute(
                "AllGather", Alu.bypass,
                ins=[cc_in[0].ap().opt()], outs=[cc_out[0].ap().opt()],
                replica_groups=[list(range(CORES))])

            # deferred constant loads (needed only after AllGather #1)
            W2 = cpool.tile([F1, F2], dt)
            nc.sync.dma_start(W2[:], p_W2[:])
            b1b = cpool.tile([128, F1], dt)
            nc.sync.dma_start(b1b[:], p_b1[:])
            b2b = cpool.tile([128, F2], dt)
            nc.sync.dma_start(b2b[:], p_b2[:])
            iota = cpool.tile([128, 128], dt)
            nc.sync.dma_start(iota[:], p_iota[:])
            ident = cpool.tile([128, 128], dt)
            nc.sync.dma_start(ident[:], p_ident[:])
            idx_sb = []
            dl_sb = []
            for h in (0, 1):
                isb = cpool.tile([128, nchunks[h] * (CHUNK // 16)], mybir.dt.int16,
                                 name=f"isb{h}")
                nc.sync.dma_start(isb[:], p_idx[h][:])
                idx_sb.append(isb)
                dsb = cpool.tile([128, nblocks[h]], dt, name=f"dsb{h}")
                nc.sync.dma_start(dsb[:], p_dl[h][:])
                dl_sb.append(dsb)

            def do_layer(li, table, self_tab, tail_fn):
                halves = [table[0:HALF, :], table[HALF:2 * HALF, :]]
                emitted = [0, 0]
                chunks = [{}, {}]

                def ensure_chunk(h, c):
                    while emitted[h] <= min(c + 3, nchunks[h] - 1):
                        ce = emitted[h]
                        st = stpool.tile([128, CHUNK_BLOCKS, F1], dt,
                                         tag=f"st{h}", name=f"st_l{li}_h{h}_c{ce}")
                        cols = CHUNK // 16
                        nc.gpsimd.dma_gather(
                            st[:], halves[h], idx_sb[h][:, ce * cols:(ce + 1) * cols],
                            CHUNK, CHUNK, F1)
                        chunks[h][ce] = st
                        emitted[h] += 1
                    return chunks[h][c]

                for t in range(NT):
                    nb = int(B[t, 0] + B[t, 1])
                    pagg = ppool.tile([128, F1], dt, tag="agg1", name=f"pg{li}_{t}")
                    k = 0
                    for h in (0, 1):
                        for b in range(int(B[t, h])):
                            gb = int(startgb[t, h]) + b
                            c, slot = gb // CHUNK_BLOCKS, gb % CHUNK_BLOCKS
                            st = ensure_chunk(h, c)
                            S = spool.tile([128, 128], dt, tag="S",
                                           name=f"S{li}_{t}_{h}_{b}")
                            nc.vector.tensor_scalar(
                                S[:], iota[:], dl_sb[h][:, gb:gb + 1], None,
                                Alu.is_equal)
                            nc.tensor.matmul(pagg[:], S[:], st[:, slot, :],
                                             start=(k == 0), stop=(k == nb - 1))
                            k += 1
                    tail_fn(t, pagg)

            # ---- layer 1
            T2sh = cpool.tile([128, NT * F1], dt)

            def tail1(t, pagg):
                e0 = wpool.tile([128, F1], dt, tag="e0", name=f"e0_{t}")
                nc.vector.tensor_tensor(out=e0[:], in0=pagg[:],
                                        in1=Tsh[:, F1 * t:F1 * (t + 1)], op=Alu.add)
                e1 = wpool.tile([128, F1], dt, tag="e1", name=f"e1_{t}")
                nc.vector.tensor_scalar(e1[:], e0[:], dinv[:, t:t + 1], None,
                                        Alu.mult)
                e2 = wpool.tile([128, F1], dt, tag="e2", name=f"e2_{t}")
                nc.vector.tensor_tensor(out=e2[:], in0=e1[:], in1=b1b[:], op=Alu.add)
                nc.vector.tensor_scalar(
                    T2sh[:, F1 * t:F1 * (t + 1)], e2[:], 0.0, dinvm[:, t:t + 1],
                    Alu.max, Alu.mult)
                nc.sync.dma_start(cc_in[1][BLK * t:BLK * (t + 1), :],
                                  T2sh[:, F1 * t:F1 * (t + 1)])

            do_layer(0, cc_out[0], None, tail1)
            nc.gpsimd.collective_compute(
                "AllGather", Alu.bypass,
                ins=[cc_in[1].ap().opt()], outs=[cc_out[1].ap().opt()],
                replica_groups=[list(range(CORES))])

            # ---- layer 2

            def tail2(t, pagg):
                a0 = wpool.tile([128, F1], dt, tag="a0", name=f"a0_{t}")
                nc.vector.tensor_tensor(out=a0[:], in0=pagg[:],
                                        in1=T2sh[:, F1 * t:F1 * (t + 1)],
                                        op=Alu.add)
                pt = popool.tile([F1, BLK], dt, tag="pt", name=f"pt_{t}")
                nc.tensor.matmul(pt[:], a0[:], ident[:], start=True, stop=True)
                aggS = wpool.tile([F1, BLK], dt, tag="aggS", name=f"as_{t}")
                nc.vector.tensor_copy(aggS[:], pt[:])
                po = popool.tile([128, F2], dt, tag="po", name=f"po_{t}")
                nc.tensor.matmul(po[:], aggS[:], W2[:], start=True, stop=True)
                e3 = wpool.tile([128, F2], dt, tag="e3", name=f"e3_{t}")
                nc.vector.tensor_scalar(e3[:], po[:], dinv[:, t:t + 1], None,
                                        Alu.mult)
                e4 = wpool.tile([128, F2], dt, tag="e4", name=f"e4_{t}")
                nc.vector.tensor_tensor(out=e4[:], in0=e3[:], in1=b2b[:], op=Alu.add)
                m = wpool.tile([128, 1], dt, tag="m", name=f"m_{t}")
                nc.vector.tensor_reduce(m[:], e4[:], axis=mybir.AxisListType.X,
                                        op=Alu.max)
                nm = wpool.tile([128, 1], dt, tag="nm", name=f"nm_{t}")
                nc.vector.tensor_scalar(nm[:], m[:], -1.0, None, Alu.mult)
                ex = wpool.tile([128, F2], dt, tag="ex", name=f"ex_{t}")
                nc.scalar.activation(ex[:], e4[:], Act.Exp, bias=nm[:, 0:1])
                sm = wpool.tile([128, 1], dt, tag="sm", name=f"sm_{t}")
                nc.vector.tensor_reduce(sm[:], ex[:], axis=mybir.AxisListType.X,
                                        op=Alu.add)
                lg = wpool.tile([128, 1], dt, tag="lg", name=f"lg_{t}")
                nc.scalar.activation(lg[:], sm[:], Act.Ln)
                fo = wpool.tile([128, F2], dt, tag="fo", name=f"fo_{t}")
                nc.vector.tensor_scalar(
                    fo[:], e4[:], m[:, 0:1], lg[:, 0:1],
                    Alu.subtract, Alu.subtract)
                nc.sync.dma_start(p_out[:, F2 * t:F2 * (t + 1)], fo[:])

            def tail2_scaled(t, pagg):
                # pagg already includes the transposed self term; note the
                # dinv scale applies AFTER adding self (both are pre-scaled
                # table rows), matching tail2's e3 step.
                tail2(t, pagg)

            do_layer(1, cc_out[1], T2sh, tail2_scaled)

    nc.finalize()
    return nc


LAST_EXEC_NS = None


def kernel(x, edge_index, W1, b1, W2, b2):
    from concourse.bass_utils import run_bass_kernel_spmd

    x = np.asarray(x, np.float32)
    data, consts, meta, perm = host_prep(x, np.asarray(edge_index), W1, b1, W2, b2)
    nc = build_nc(meta)
    in_maps = []
    for i in range(CORES):
        m = dict(data[i])
        m.update({k: np.ascontiguousarray(v) for k, v in consts.items()})
        in_maps.append(m)
    import os as _os
    trace = bool(int(_os.environ.get("GCN_TRACE", "0")))
    res = run_bass_kernel_spmd(nc, in_maps, core_ids=list(range(CORES)), trace=trace)
    global LAST_EXEC_NS
    LAST_EXEC_NS = res.exec_time_ns
    outs = []
    for i in range(CORES):
        o = res.results[i]["out"]  # [128, NT*F2]
        outs.append(o.reshape(128, NT, F2).transpose(1, 0, 2).reshape(SHP, F2))
    return assemble_output(np.stack(outs), perm)


if __name__ == "__main__":
    import reference
    inputs = {k: np.asarray(v) for k, v in reference.setup_inputs().items()}
    expected = np.asarray(reference.reference(**{k: v for k, v in inputs.items()}))
    data, consts, meta, perm = host_prep(**inputs)
    print("B max:", meta["B"].max(), "nblocks:", meta["nblocks"],
          "nchunks:", meta["nchunks"])
    outs = numpy_device_sim(data, consts, meta)
    got = assemble_output(outs, perm)
    err = np.abs(got - expected)
    rel = err.max() / np.abs(expected).max()
    print(f"numpy-sim max abs err {err.max():.3e}  rel {rel:.3e}")
